# revision 30
# baseline (speedup 1.0000x reference)
"""Trainium2 Bass kernel for pre-LN multi-head self-attention.

Module: y = LN(x); qkv = y @ w_qkv; attention(8 heads, dh=64); out = ao @ w_out
Shapes: x [4, 2048, 512], w_qkv [512, 1536], w_out [512, 512], fp32.

Sharding (8 cores): core c -> batch b = c//2, head-group g = c%2 (4 heads).
Each core computes LN + QKV (its head slice) + attention + a partial output
projection (its heads' rows of w_out); the host sums the two partials per batch.

Design (v2, ACT-exp-stream centric):
  The softmax exp stream on the Scalar/ACT engine (16.8M elems/core at
  1 elem/cycle/lane @1.2GHz ~= 128us) is the hard floor; everything else is
  scheduled to keep that stream airtight and the PE clock warm (HAM K=8/8).
  - LN phase: 16-deep x-tile DMA lookahead; rstd = exp(-0.5*ln(var+eps)) so
    the whole kernel uses ONE ACT table set (natural_log_exp); y-affine on
    ACT, PSUM evictions on DVE; V-projection matmuls pipelined per token
    group to keep the PE busy during LN.
  - QK projections use [128,1024] PSUM accumulators from the same pool that
    later serves the score tiles; the j=1 head-pair projections are
    interleaved into early stage D so exps start right after j=0.
  - Stage D: depth-2 software pipeline (scores i+2 issue before attn@V i);
    single ao accumulator (eviction hides under the next unit's exp latency);
    softmax denominators (ones-column of V) broadcast across partitions via
    gpsimd.partition_broadcast and applied with a DVE divide -- no DRAM
    roundtrip, no 1-partition reciprocals.
  - Output projection tiles are spread one-per-item into PE slack; the last
    unit normalizes in 128-col chunks interleaved with the final tiles.
"""

import sys

if "/opt/trn_rl_repo" not in sys.path:
    sys.path.insert(0, "/opt/trn_rl_repo")

from contextlib import ExitStack

import numpy as np

import concourse.bass as bass
import concourse.tile as tile
from concourse.masks import make_identity
from concourse import bacc, mybir
from concourse.bass_utils import run_bass_kernel_spmd

B, N, D = 4, 2048, 512
H, DH = 8, 64
HPC = 4                 # heads per core
FPC = HPC * DH          # 256 features per core
P = 128
NT = N // P             # 16 token tiles
DT = D // P             # 4 d tiles
EPS = 1e-6
SCALE = DH ** -0.5
F32 = mybir.dt.float32
BF16 = mybir.dt.bfloat16
ALU = mybir.AluOpType
AFT = mybir.ActivationFunctionType
QH = 1024               # q-half width (stage D unit = (qh, h))


def build_kernel():
    nc = bacc.Bacc("TRN2", target_bir_lowering=False, debug=False)
    xb = nc.dram_tensor("xb", [N, D], F32, kind="ExternalInput").ap()
    wq = nc.dram_tensor("wq", [D, FPC], BF16, kind="ExternalInput").ap()
    wk = nc.dram_tensor("wk", [D, FPC], BF16, kind="ExternalInput").ap()
    wv = nc.dram_tensor("wv", [D, FPC], BF16, kind="ExternalInput").ap()
    wo = nc.dram_tensor("wo", [FPC, D], BF16, kind="ExternalInput").ap()
    bq = nc.dram_tensor("bq", [FPC], F32, kind="ExternalInput").ap()
    bk = nc.dram_tensor("bk", [FPC], F32, kind="ExternalInput").ap()
    bv = nc.dram_tensor("bv", [FPC], F32, kind="ExternalInput").ap()
    out = nc.dram_tensor("out", [N, D], F32, kind="ExternalOutput").ap()

    with tile.TileContext(nc, pool_alloc_mode="queue") as tc, ExitStack() as ctx:
        consts = ctx.enter_context(tc.tile_pool(name="consts", bufs=1))
        big = ctx.enter_context(tc.tile_pool(name="big", bufs=1))

        identity = consts.tile([P, P], BF16)
        make_identity(nc, identity)
        eps_t = consts.tile([P, 1], F32)
        nc.vector.memset(eps_t, EPS)

        yT = [big.tile([P, N], BF16, tag=f"yT{j}", name=f"yT{j}") for j in range(DT)]
        qT = [big.tile([P, N], BF16, tag=f"qT{j}", name=f"qT{j}") for j in range(2)]
        kT = [big.tile([P, N], BF16, tag=f"kT{j}", name=f"kT{j}") for j in range(2)]
        aoT = [big.tile([P, N], BF16, tag=f"aoT{j}", name=f"aoT{j}") for j in range(2)]
        v_sb = big.tile([P, NT, HPC, DH + 1], BF16)
        ones_col = consts.tile([P, 1], F32)
        nc.vector.memset(ones_col, 1.0)
        nc.vector.tensor_copy(
            v_sb[:, :, :, DH : DH + 1],
            ones_col[:, 0:1].to_broadcast((P, NT, HPC, 1)),
        )

        # ---- input + weight DMAs: x in 4 batched group DMAs (one trigger
        # each -- the sync engine serializes triggers at ~600ns apiece)
        xin = ctx.enter_context(tc.tile_pool(name="xin", bufs=4))
        x_gs = []
        for ig in range(4):
            x_g = xin.tile([P, 4, D], F32, tag="xg", name=f"xg{ig}")
            if ig == 0:
                # split the first group so tile 0 lands fast (small trigger)
                nc.sync.dma_start(
                    out=x_g[:, 0:1, :],
                    in_=xb[0:P, :].rearrange("(t p) d -> p t d", p=P),
                )
                nc.sync.dma_start(
                    out=x_g[:, 1:4, :],
                    in_=xb[P : 4 * P, :].rearrange("(t p) d -> p t d", p=P),
                )
            else:
                nc.sync.dma_start(
                    out=x_g,
                    in_=xb[ig * 512 : (ig + 1) * 512, :].rearrange(
                        "(t p) d -> p t d", p=P
                    ),
                )
            x_gs.append(x_g)
            if ig == 0:
                w_v_sb = consts.tile([P, DT, FPC], BF16)
                nc.sync.dma_start(
                    out=w_v_sb, in_=wv.rearrange("(t p) f -> p t f", p=P)
                )
                bv_b = consts.tile([P, FPC], F32)
                bv_bcast = bass.AP(
                    tensor=bv.tensor, offset=bv.offset, ap=[[0, P]] + list(bv.ap)
                )
                nc.sync.dma_start(out=bv_b, in_=bv_bcast)
            if ig == 1:
                w_q_sb = consts.tile([P, DT, FPC], BF16)
                nc.sync.dma_start(
                    out=w_q_sb, in_=wq.rearrange("(t p) f -> p t f", p=P)
                )
                w_k_sb = consts.tile([P, DT, FPC], BF16)
                nc.sync.dma_start(
                    out=w_k_sb, in_=wk.rearrange("(t p) f -> p t f", p=P)
                )
            if ig == 2:
                bq_sb = consts.tile([P, 2], F32)
                nc.sync.dma_start(out=bq_sb, in_=bq.rearrange("(t p) -> p t", p=P))
                bk_sb = consts.tile([P, 2], F32)
                nc.sync.dma_start(out=bk_sb, in_=bk.rearrange("(t p) -> p t", p=P))
                w_o_sb = consts.tile([P, 2, D], BF16)
                nc.sync.dma_start(
                    out=w_o_sb, in_=wo.rearrange("(t p) f -> p t f", p=P)
                )

        # bigp serves the j=0 QK accumulators and the stage-D score tiles
        bigp = ctx.enter_context(
            tc.tile_pool(name="bigp", bufs=2, space="PSUM")
        )

        def qk_half(w_sb, b_sb, dstT, j, half, on_act):
            ps = bigp.tile([P, QH], F32, tag="mm", name=f"qk{j}_{half}_{dstT is qT}")
            for dt in range(DT):
                for c in range(2):
                    mm = nc.tensor.matmul(
                        ps[:, c * 512 : (c + 1) * 512],
                        lhsT=(w_sb[:, dt, j * P : (j + 1) * P]),
                        rhs=(yT[dt][:, half * QH + c * 512 : half * QH + (c + 1) * 512]),
                        start=(dt == 0),
                        stop=(dt == DT - 1),
                    )
                    if c == 1:
                        mm.ins.ldweights = False
            cols = slice(half * QH, (half + 1) * QH)
            if on_act:
                nc.scalar.activation(
                    out=dstT[j][:, cols], in_=ps, func=AFT.Identity,
                    bias=b_sb[:, j : j + 1],
                )
            else:
                nc.vector.tensor_scalar(
                    out=dstT[j][:, cols], in0=ps, scalar1=b_sb[:, j : j + 1],
                    scalar2=None, op0=ALU.add,
                )

        # ---- Phase A: LayerNorm + transpose + V projection, pipelined;
        # the j=0 half-0 Q/K projections fire as soon as groups 0/1 land ----
        with tc.tile_pool(name="ln", bufs=8) as ln, tc.tile_pool(
            name="tp_psum", bufs=2, space="PSUM"
        ) as tpp, tc.tile_pool(name="v_psum", bufs=2, space="PSUM") as vpp:
            # dep-free PE warmup: dummy matmuls so the HAM clock-gate opens
            # (K=8/8) before the real prefix work arrives
            dmy = ln.tile([P, 512], BF16, tag="dmy")
            nc.vector.memset(dmy, 0.0)
            for k in range(24):
                dps = vpp.tile([P, FPC], F32, tag="v", name=f"dmy{k}")
                nc.tensor.matmul(
                    dps, lhsT=identity, rhs=dmy[:, 0:FPC], start=True, stop=True
                )
            for ig in range(NT // 4):  # groups of 4 token tiles
                y_ts = []
                for ii in range(4):
                    i = ig * 4 + ii
                    x_t = x_gs[ig][:, ii, :]
                    stats = ln.tile([P, 6], F32, tag="stats")
                    nc.vector.bn_stats(out=stats, in_=x_t)
                    mv = ln.tile([P, 2], F32, tag="mv")
                    nc.vector.bn_aggr(out=mv, in_=stats)
                    std = ln.tile([P, 1], F32, tag="std")
                    nc.scalar.activation(
                        out=std, in_=mv[:, 1:2], func=AFT.Sqrt, bias=eps_t[:, 0:1]
                    )
                    rstd = ln.tile([P, 1], F32, tag="rstd")
                    nc.vector.reciprocal(out=rstd, in_=std)
                    nmr = ln.tile([P, 1], F32, tag="nmr")
                    nc.vector.tensor_scalar(
                        out=nmr,
                        in0=mv[:, 0:1],
                        scalar1=rstd[:, 0:1],
                        scalar2=-1.0,
                        op0=ALU.mult,
                        op1=ALU.mult,
                    )
                    y_t = ln.tile([P, D], BF16, tag="y", bufs=6)
                    nc.scalar.activation(
                        out=y_t,
                        in_=x_t,
                        func=AFT.Identity,
                        scale=rstd[:, 0:1],
                        bias=nmr[:, 0:1],
                    )
                    y_ts.append(y_t)
                for j in range(DT):
                    pt = tpp.tile([P, 512], BF16, tag="tp")
                    for ii in range(4):
                        nc.tensor.transpose(
                            pt[:, ii * P : (ii + 1) * P],
                            y_ts[ii][:, j * P : (j + 1) * P],
                            identity,
                        )
                    if j < 2:
                        nc.vector.tensor_copy(
                            yT[j][:, ig * 512 : (ig + 1) * 512], pt
                        )
                    else:
                        nc.scalar.copy(
                            yT[j][:, ig * 512 : (ig + 1) * 512], pt
                        )
                # V projection for this group's 4 token tiles
                for ii in range(4):
                    i = ig * 4 + ii
                    ps = vpp.tile([P, FPC], F32, tag="v", name=f"v{i}")
                    for dt in range(DT):
                        nc.tensor.matmul(
                            ps,
                            lhsT=(yT[dt][:, i * P : (i + 1) * P]),
                            rhs=(w_v_sb[:, dt, :]),
                            start=(dt == 0),
                            stop=(dt == DT - 1),
                        )
                    nc.vector.tensor_tensor(
                        out=v_sb[:, i, :, 0:DH],
                        in0=ps.rearrange("p (h d) -> p h d", h=HPC),
                        in1=bv_b.rearrange("p (h d) -> p h d", h=HPC),
                        op=ALU.add,
                    )
                if ig == 1:
                    # token half 0 of yT is complete: fire the j=0 half-0
                    # Q/K projections so stage D can start ASAP
                    qk_half(w_k_sb, bk_sb, kT, 0, 0, on_act=True)
                    qk_half(w_q_sb, bq_sb, qT, 0, 0, on_act=True)

        # remaining projection work in [128,512] chunks through the o_psum
        # pool (idle until the first out-projection), interleaved into early
        # stage D: j=0 half-1 first (needed at kb>=8), then j=1 (heads 2/3)
        qk_work = (
            [(w_k_sb, bk_sb, kT, 0, nt) for nt in (2, 3)]
            + [(w_k_sb, bk_sb, kT, 1, nt) for nt in range(4)]
            + [(w_q_sb, bq_sb, qT, 1, nt) for nt in range(4)]
            + [(w_q_sb, bq_sb, qT, 0, nt) for nt in (2, 3)]
        )

        # ---- Stage D ----
        with tc.tile_pool(name="ao_psum", bufs=1, space="PSUM") as aop, tc.tile_pool(
            name="exp_sb", bufs=6
        ) as exps, tc.tile_pool(name="nrm", bufs=3) as nrm, tc.tile_pool(
            name="o_psum", bufs=2, space="PSUM"
        ) as opp, tc.tile_pool(name="o_sb", bufs=3) as osb:
            items = [
                (qh, h, kb) for qh in range(2) for h in range(HPC) for kb in range(NT)
            ]
            ex_tiles = {}
            ao_tiles = {}

            def sc_exp(i):
                qh, h, kb = items[i]
                j, po = h // 2, (h % 2) * DH
                q0 = qh * QH
                sc = bigp.tile([P, QH], F32, tag="mm", name=f"sc{i}")
                for c in range(2):
                    mm = nc.tensor.matmul(
                        sc[:, c * 512 : (c + 1) * 512],
                        lhsT=(kT[j][po : po + DH, kb * P : (kb + 1) * P]),
                        rhs=(qT[j][po : po + DH, q0 + c * 512 : q0 + (c + 1) * 512]),
                        start=True,
                        stop=True,
                    )
                    if c == 1:
                        mm.ins.ldweights = False
                ex = exps.tile([P, QH], BF16, tag="ex", name=f"ex{i}")
                nc.scalar.activation(out=ex, in_=sc, func=AFT.Exp, scale=SCALE)
                ex_tiles[i] = ex

            def normalize(i, qh, h, j, po, q0, nchunk):
                # evict the unnormalized accumulator (frees the PSUM bank),
                # broadcast the denominator row across partitions on gpsimd,
                # then divide on DVE
                ao_ps = ao_tiles.pop((qh, h))
                ao_sb = nrm.tile([DH + 1, QH], F32, tag="ao_sb", name=f"aosb{i}")
                nc.vector.tensor_copy(ao_sb, ao_ps)
                cw = QH // nchunk
                stage_ts = [
                    osb.tile([P, 4, D], F32, tag="stg", name=f"stg{i}_{s}", bufs=2)
                    for s in range(nchunk // 4)
                ] if nchunk > 1 else []
                # stage the denominator row onto partition 0: the fast
                # 1-cyc/elem custom-DVE reciprocal reads partition 0 only
                # (standard reciprocal is ~7 cyc/elem)
                dn = nrm.tile([1, QH], F32, tag="dn", name=f"dn{i}", bufs=2)
                nc.vector.tensor_copy(dn, ao_sb[DH : DH + 1, :])
                for ch in range(nchunk):
                    cs = ch * cw
                    recip = nrm.tile([1, QH], F32, tag="rc", name=f"rc{i}_{ch}", bufs=2)
                    nc.vector.reciprocal_approx_fast(
                        out=recip[:, 0:cw], in_=dn[0:1, cs : cs + cw]
                    )
                    rb = nrm.tile([DH, QH], F32, tag="rb", name=f"rb{i}_{ch}", bufs=2)
                    nc.gpsimd.partition_broadcast(
                        rb[:, 0:cw], recip[:, 0:cw], channels=DH
                    )
                    nc.vector.tensor_tensor(
                        out=aoT[j][po : po + DH, q0 + cs : q0 + cs + cw],
                        in0=ao_sb[0:DH, cs : cs + cw],
                        in1=rb[:, 0:cw],
                        op=ALU.mult,
                    )
                    if nchunk > 1:
                        st = stage_ts[ch // 4]
                        outproj_tile(NT // 2 + ch, stage=(st, ch % 4))

            def attn_v(i):
                qh, h, kb = items[i]
                j, po = h // 2, (h % 2) * DH
                q0 = qh * QH
                if kb == 0:
                    ao_tiles[(qh, h)] = aop.tile(
                        [DH + 1, QH], F32, tag="ao", name=f"ao{qh}_{h}"
                    )
                ao_ps = ao_tiles[(qh, h)]
                ex = ex_tiles.pop(i)
                for c in range(2):
                    mm = nc.tensor.matmul(
                        ao_ps[:, c * 512 : (c + 1) * 512],
                        lhsT=(v_sb[:, kb, h, :]),
                        rhs=(ex[:, c * 512 : (c + 1) * 512]),
                        start=(kb == 0),
                        stop=(kb == NT - 1),
                    )
                    if c == 1:
                        mm.ins.ldweights = False
                if kb == NT - 1:
                    normalize(i, qh, h, j, po, q0, 8 if i == len(items) - 1 else 1)

            def outproj_tile(mt, stage=None):
                ps = opp.tile([P, D], F32, tag="o", name=f"o{mt}")
                for kt in range(2):
                    nc.tensor.matmul(
                        ps,
                        lhsT=(aoT[kt][:, mt * P : (mt + 1) * P]),
                        rhs=(w_o_sb[:, kt, :]),
                        start=(kt == 0),
                        stop=(kt == 1),
                    )
                if stage is None:
                    ot = osb.tile([P, D], F32, tag="ot", name=f"ot{mt}")
                    nc.vector.tensor_copy(ot, ps)
                    nc.sync.dma_start(out=out[mt * P : (mt + 1) * P, :], in_=ot)
                else:
                    # stage 4 tiles, then one batched DMA (fewer sync-engine
                    # trigger serializations in the kernel tail)
                    st, si = stage
                    nc.vector.tensor_copy(st[:, si, :], ps)
                    if si == 3:
                        m0 = mt - 3
                        nc.sync.dma_start(
                            out=out[m0 * P : (m0 + 4) * P, :].rearrange(
                                "(t p) d -> p t d", p=P
                            ),
                            in_=st,
                        )

            def qk_chunk(w_sb, b_sb, dstT, j, nt):
                ps = opp.tile([P, 512], F32, tag="o", name=f"qkc{j}_{dstT is qT}_{nt}")
                for dt in range(DT):
                    nc.tensor.matmul(
                        ps,
                        lhsT=(w_sb[:, dt, j * P : (j + 1) * P]),
                        rhs=(yT[dt][:, nt * 512 : (nt + 1) * 512]),
                        start=(dt == 0),
                        stop=(dt == DT - 1),
                    )
                nc.vector.tensor_scalar(
                    out=dstT[j][:, nt * 512 : (nt + 1) * 512],
                    in0=ps, scalar1=b_sb[:, j : j + 1], scalar2=None, op0=ALU.add,
                )

            # item index -> extra PE work issued right after attn_v(i):
            # deferred QK chunks early, first out-projection half spread
            # after unit (0,3)'s normalize completes
            post = {}
            for n, w in enumerate(qk_work):
                post[3 * n] = ("qk", w)
            for mt in range(NT // 2):
                post[76 + 2 * mt] = ("op", mt)

            DEPTH = 3
            for i in range(min(DEPTH, len(items))):
                sc_exp(i)
            for i in range(len(items)):
                if i + DEPTH < len(items):
                    sc_exp(i + DEPTH)
                attn_v(i)
                extra = post.get(i)
                if extra is not None:
                    if extra[0] == "qk":
                        qk_chunk(*extra[1])
                    else:
                        outproj_tile(extra[1])

    nc.compile()
    return nc


_NC_CACHE = None
_LAST_RESULT = None


def kernel(x, ln_scale, ln_bias, w_qkv, w_out):
    global _NC_CACHE, _LAST_RESULT
    if _NC_CACHE is None:
        _NC_CACHE = build_kernel()
    nc = _NC_CACHE

    import ml_dtypes

    x = np.asarray(x, np.float32)
    w_eff = (np.asarray(ln_scale, np.float32)[:, None] * np.asarray(w_qkv, np.float32))
    b_row = np.asarray(ln_bias, np.float32) @ np.asarray(w_qkv, np.float32)
    w_eff = w_eff.astype(ml_dtypes.bfloat16)
    w_out = np.asarray(w_out, np.float32).astype(ml_dtypes.bfloat16)

    in_maps = []
    for c in range(8):
        b, g = c // 2, c % 2
        s = slice(FPC * g, FPC * g + FPC)
        ks = slice(512 + FPC * g, 512 + FPC * g + FPC)
        vs = slice(1024 + FPC * g, 1024 + FPC * g + FPC)
        in_maps.append(
            {
                "xb": np.ascontiguousarray(x[b]),
                "wq": np.ascontiguousarray(w_eff[:, s]),
                "wk": np.ascontiguousarray(w_eff[:, ks]),
                "wv": np.ascontiguousarray(w_eff[:, vs]),
                "wo": np.ascontiguousarray(w_out[s, :]),
                "bq": np.ascontiguousarray(b_row[s]),
                "bk": np.ascontiguousarray(b_row[ks]),
                "bv": np.ascontiguousarray(b_row[vs]),
            }
        )
    res = run_bass_kernel_spmd(nc, in_maps, core_ids=list(range(8)))
    _LAST_RESULT = res
    outs = [res.results[c]["out"] for c in range(8)]
    return np.stack([outs[2 * b] + outs[2 * b + 1] for b in range(B)]).astype(
        np.float32
    )


if __name__ == "__main__":
    xs = np.random.randn(B, N, D).astype(np.float32)
    o = kernel(
        x=xs,
        ln_scale=np.ones(D, np.float32),
        ln_bias=np.zeros(D, np.float32),
        w_qkv=(np.random.randn(D, 3 * H * DH) / np.sqrt(D)).astype(np.float32),
        w_out=(np.random.randn(H * DH, D) / np.sqrt(H * DH)).astype(np.float32),
    )
    print(o.shape, o.dtype)


# revision 31
# speedup vs baseline: 1.0051x; 1.0051x over previous
"""Trainium2 Bass kernel for pre-LN multi-head self-attention.

Module: y = LN(x); qkv = y @ w_qkv; attention(8 heads, dh=64); out = ao @ w_out
Shapes: x [4, 2048, 512], w_qkv [512, 1536], w_out [512, 512], fp32.

Sharding (8 cores): core c -> batch b = c//2, head-group g = c%2 (4 heads).
Each core computes LN + QKV (its head slice) + attention + a partial output
projection (its heads' rows of w_out); the host sums the two partials per batch.

Design (v2, ACT-exp-stream centric):
  The softmax exp stream on the Scalar/ACT engine (16.8M elems/core at
  1 elem/cycle/lane @1.2GHz ~= 128us) is the hard floor; everything else is
  scheduled to keep that stream airtight and the PE clock warm (HAM K=8/8).
  - LN phase: 16-deep x-tile DMA lookahead; rstd = exp(-0.5*ln(var+eps)) so
    the whole kernel uses ONE ACT table set (natural_log_exp); y-affine on
    ACT, PSUM evictions on DVE; V-projection matmuls pipelined per token
    group to keep the PE busy during LN.
  - QK projections use [128,1024] PSUM accumulators from the same pool that
    later serves the score tiles; the j=1 head-pair projections are
    interleaved into early stage D so exps start right after j=0.
  - Stage D: depth-2 software pipeline (scores i+2 issue before attn@V i);
    single ao accumulator (eviction hides under the next unit's exp latency);
    softmax denominators (ones-column of V) broadcast across partitions via
    gpsimd.partition_broadcast and applied with a DVE divide -- no DRAM
    roundtrip, no 1-partition reciprocals.
  - Output projection tiles are spread one-per-item into PE slack; the last
    unit normalizes in 128-col chunks interleaved with the final tiles.
"""

import sys

if "/opt/trn_rl_repo" not in sys.path:
    sys.path.insert(0, "/opt/trn_rl_repo")

from contextlib import ExitStack

import numpy as np

import concourse.bass as bass
import concourse.tile as tile
from concourse.masks import make_identity
from concourse import bacc, mybir
from concourse.bass_utils import run_bass_kernel_spmd

B, N, D = 4, 2048, 512
H, DH = 8, 64
HPC = 4                 # heads per core
FPC = HPC * DH          # 256 features per core
P = 128
NT = N // P             # 16 token tiles
DT = D // P             # 4 d tiles
EPS = 1e-6
SCALE = DH ** -0.5
F32 = mybir.dt.float32
BF16 = mybir.dt.bfloat16
ALU = mybir.AluOpType
AFT = mybir.ActivationFunctionType
QH = 1024               # q-half width (stage D unit = (qh, h))


def build_kernel():
    nc = bacc.Bacc("TRN2", target_bir_lowering=False, debug=False)
    xb = nc.dram_tensor("xb", [N, D], F32, kind="ExternalInput").ap()
    wq = nc.dram_tensor("wq", [D, FPC], BF16, kind="ExternalInput").ap()
    wk = nc.dram_tensor("wk", [D, FPC], BF16, kind="ExternalInput").ap()
    wv = nc.dram_tensor("wv", [D, FPC], BF16, kind="ExternalInput").ap()
    wo = nc.dram_tensor("wo", [FPC, D], BF16, kind="ExternalInput").ap()
    bq = nc.dram_tensor("bq", [FPC], F32, kind="ExternalInput").ap()
    bk = nc.dram_tensor("bk", [FPC], F32, kind="ExternalInput").ap()
    bv = nc.dram_tensor("bv", [FPC], F32, kind="ExternalInput").ap()
    out = nc.dram_tensor("out", [N, D], F32, kind="ExternalOutput").ap()

    with tile.TileContext(nc, pool_alloc_mode="queue") as tc, ExitStack() as ctx:
        consts = ctx.enter_context(tc.tile_pool(name="consts", bufs=1))
        big = ctx.enter_context(tc.tile_pool(name="big", bufs=1))

        identity = consts.tile([P, P], BF16)
        make_identity(nc, identity)
        eps_t = consts.tile([P, 1], F32)
        nc.vector.memset(eps_t, EPS)

        yT = [big.tile([P, N], BF16, tag=f"yT{j}", name=f"yT{j}") for j in range(DT)]
        qT = [big.tile([P, N], BF16, tag=f"qT{j}", name=f"qT{j}") for j in range(2)]
        kT = [big.tile([P, N], BF16, tag=f"kT{j}", name=f"kT{j}") for j in range(2)]
        aoT = [big.tile([P, N], BF16, tag=f"aoT{j}", name=f"aoT{j}") for j in range(2)]
        v_sb = big.tile([P, NT, HPC, DH + 1], BF16)
        ones_col = consts.tile([P, 1], F32)
        nc.vector.memset(ones_col, 1.0)
        nc.vector.tensor_copy(
            v_sb[:, :, :, DH : DH + 1],
            ones_col[:, 0:1].to_broadcast((P, NT, HPC, 1)),
        )

        # ---- input + weight DMAs: x in 4 batched group DMAs (one trigger
        # each -- the sync engine serializes triggers at ~600ns apiece)
        xin = ctx.enter_context(tc.tile_pool(name="xin", bufs=4))
        x_gs = []
        for ig in range(4):
            x_g = xin.tile([P, 4, D], F32, tag="xg", name=f"xg{ig}")
            if ig == 0:
                # split the first group so tile 0 lands fast (small trigger)
                nc.sync.dma_start(
                    out=x_g[:, 0:1, :],
                    in_=xb[0:P, :].rearrange("(t p) d -> p t d", p=P),
                )
                nc.sync.dma_start(
                    out=x_g[:, 1:4, :],
                    in_=xb[P : 4 * P, :].rearrange("(t p) d -> p t d", p=P),
                )
            else:
                nc.sync.dma_start(
                    out=x_g,
                    in_=xb[ig * 512 : (ig + 1) * 512, :].rearrange(
                        "(t p) d -> p t d", p=P
                    ),
                )
            x_gs.append(x_g)
            if ig == 0:
                w_v_sb = consts.tile([P, DT, FPC], BF16)
                nc.sync.dma_start(
                    out=w_v_sb, in_=wv.rearrange("(t p) f -> p t f", p=P)
                )
                bv_b = consts.tile([P, FPC], F32)
                bv_bcast = bass.AP(
                    tensor=bv.tensor, offset=bv.offset, ap=[[0, P]] + list(bv.ap)
                )
                nc.sync.dma_start(out=bv_b, in_=bv_bcast)
            if ig == 1:
                w_q_sb = consts.tile([P, DT, FPC], BF16)
                nc.sync.dma_start(
                    out=w_q_sb, in_=wq.rearrange("(t p) f -> p t f", p=P)
                )
                w_k_sb = consts.tile([P, DT, FPC], BF16)
                nc.sync.dma_start(
                    out=w_k_sb, in_=wk.rearrange("(t p) f -> p t f", p=P)
                )
            if ig == 2:
                bq_sb = consts.tile([P, 2], F32)
                nc.sync.dma_start(out=bq_sb, in_=bq.rearrange("(t p) -> p t", p=P))
                bk_sb = consts.tile([P, 2], F32)
                nc.sync.dma_start(out=bk_sb, in_=bk.rearrange("(t p) -> p t", p=P))
                w_o_sb = consts.tile([P, 2, D], BF16)
                nc.sync.dma_start(
                    out=w_o_sb, in_=wo.rearrange("(t p) f -> p t f", p=P)
                )

        # bigp serves the j=0 QK accumulators and the stage-D score tiles
        bigp = ctx.enter_context(
            tc.tile_pool(name="bigp", bufs=2, space="PSUM")
        )

        def qk_half(w_sb, b_sb, dstT, j, half, on_act):
            ps = bigp.tile([P, QH], F32, tag="mm", name=f"qk{j}_{half}_{dstT is qT}")
            for dt in range(DT):
                for c in range(2):
                    mm = nc.tensor.matmul(
                        ps[:, c * 512 : (c + 1) * 512],
                        lhsT=(w_sb[:, dt, j * P : (j + 1) * P]),
                        rhs=(yT[dt][:, half * QH + c * 512 : half * QH + (c + 1) * 512]),
                        start=(dt == 0),
                        stop=(dt == DT - 1),
                    )
                    if c == 1:
                        mm.ins.ldweights = False
            cols = slice(half * QH, (half + 1) * QH)
            if on_act:
                nc.scalar.activation(
                    out=dstT[j][:, cols], in_=ps, func=AFT.Identity,
                    bias=b_sb[:, j : j + 1],
                )
            else:
                nc.vector.tensor_scalar(
                    out=dstT[j][:, cols], in0=ps, scalar1=b_sb[:, j : j + 1],
                    scalar2=None, op0=ALU.add,
                )

        # ---- Phase A: LayerNorm + transpose + V projection, pipelined;
        # the j=0 half-0 Q/K projections fire as soon as groups 0/1 land ----
        with tc.tile_pool(name="ln", bufs=8) as ln, tc.tile_pool(
            name="tp_psum", bufs=2, space="PSUM"
        ) as tpp, tc.tile_pool(name="v_psum", bufs=2, space="PSUM") as vpp:
            # dep-free PE warmup: dummy matmuls so the HAM clock-gate opens
            # (K=8/8) before the real prefix work arrives
            dmy = ln.tile([P, 512], BF16, tag="dmy")
            nc.vector.memset(dmy, 0.0)
            for k in range(24):
                dps = vpp.tile([P, FPC], F32, tag="v", name=f"dmy{k}")
                nc.tensor.matmul(
                    dps, lhsT=identity, rhs=dmy[:, 0:FPC], start=True, stop=True
                )
            for ig in range(NT // 4):  # groups of 4 token tiles
                y_ts = []
                for ii in range(4):
                    i = ig * 4 + ii
                    x_t = x_gs[ig][:, ii, :]
                    stats = ln.tile([P, 6], F32, tag="stats")
                    nc.vector.bn_stats(out=stats, in_=x_t)
                    mv = ln.tile([P, 2], F32, tag="mv")
                    nc.vector.bn_aggr(out=mv, in_=stats)
                    std = ln.tile([P, 1], F32, tag="std")
                    nc.scalar.activation(
                        out=std, in_=mv[:, 1:2], func=AFT.Sqrt, bias=eps_t[:, 0:1]
                    )
                    rstd = ln.tile([P, 1], F32, tag="rstd")
                    nc.vector.reciprocal(out=rstd, in_=std)
                    nmr = ln.tile([P, 1], F32, tag="nmr")
                    nc.vector.tensor_scalar(
                        out=nmr,
                        in0=mv[:, 0:1],
                        scalar1=rstd[:, 0:1],
                        scalar2=-1.0,
                        op0=ALU.mult,
                        op1=ALU.mult,
                    )
                    y_t = ln.tile([P, D], BF16, tag="y", bufs=6)
                    nc.scalar.activation(
                        out=y_t,
                        in_=x_t,
                        func=AFT.Identity,
                        scale=rstd[:, 0:1],
                        bias=nmr[:, 0:1],
                    )
                    y_ts.append(y_t)
                for j in range(DT):
                    pt = tpp.tile([P, 512], BF16, tag="tp")
                    for ii in range(4):
                        nc.tensor.transpose(
                            pt[:, ii * P : (ii + 1) * P],
                            y_ts[ii][:, j * P : (j + 1) * P],
                            identity,
                        )
                    if j < 2:
                        nc.vector.tensor_copy(
                            yT[j][:, ig * 512 : (ig + 1) * 512], pt
                        )
                    else:
                        nc.scalar.copy(
                            yT[j][:, ig * 512 : (ig + 1) * 512], pt
                        )
                # V projection for this group's 4 token tiles
                for ii in range(4):
                    i = ig * 4 + ii
                    ps = vpp.tile([P, FPC], F32, tag="v", name=f"v{i}")
                    for dt in range(DT):
                        nc.tensor.matmul(
                            ps,
                            lhsT=(yT[dt][:, i * P : (i + 1) * P]),
                            rhs=(w_v_sb[:, dt, :]),
                            start=(dt == 0),
                            stop=(dt == DT - 1),
                        )
                    nc.vector.tensor_tensor(
                        out=v_sb[:, i, :, 0:DH],
                        in0=ps.rearrange("p (h d) -> p h d", h=HPC),
                        in1=bv_b.rearrange("p (h d) -> p h d", h=HPC),
                        op=ALU.add,
                    )
                if ig == 1:
                    # token half 0 of yT is complete: fire the j=0 half-0
                    # Q/K projections so stage D can start ASAP
                    qk_half(w_k_sb, bk_sb, kT, 0, 0, on_act=True)
                    qk_half(w_q_sb, bq_sb, qT, 0, 0, on_act=True)

        # remaining projection work in [128,512] chunks through the o_psum
        # pool (idle until the first out-projection), interleaved into early
        # stage D: j=0 half-1 first (needed at kb>=8), then j=1 (heads 2/3)
        qk_work = (
            [(w_k_sb, bk_sb, kT, 0, nt) for nt in (2, 3)]
            + [(w_k_sb, bk_sb, kT, 1, nt) for nt in range(4)]
            + [(w_q_sb, bq_sb, qT, 1, nt) for nt in range(4)]
            + [(w_q_sb, bq_sb, qT, 0, nt) for nt in (2, 3)]
        )

        # ---- Stage D ----
        with tc.tile_pool(name="ao_psum", bufs=1, space="PSUM") as aop, tc.tile_pool(
            name="exp_sb", bufs=6
        ) as exps, tc.tile_pool(name="nrm", bufs=3) as nrm, tc.tile_pool(
            name="o_psum", bufs=2, space="PSUM"
        ) as opp, tc.tile_pool(name="o_sb", bufs=3) as osb:
            items = [
                (qh, h, kb) for qh in range(2) for h in range(HPC) for kb in range(NT)
            ]
            ex_tiles = {}
            ao_tiles = {}

            def sc_exp(i):
                qh, h, kb = items[i]
                j, po = h // 2, (h % 2) * DH
                q0 = qh * QH
                sc = bigp.tile([P, QH], F32, tag="mm", name=f"sc{i}")
                for c in range(2):
                    mm = nc.tensor.matmul(
                        sc[:, c * 512 : (c + 1) * 512],
                        lhsT=(kT[j][po : po + DH, kb * P : (kb + 1) * P]),
                        rhs=(qT[j][po : po + DH, q0 + c * 512 : q0 + (c + 1) * 512]),
                        start=True,
                        stop=True,
                    )
                    if c == 1:
                        mm.ins.ldweights = False
                ex = exps.tile([P, QH], BF16, tag="ex", name=f"ex{i}")
                nc.scalar.activation(out=ex, in_=sc, func=AFT.Exp, scale=SCALE)
                ex_tiles[i] = ex

            def normalize(i, qh, h, j, po, q0, nchunk):
                # evict the unnormalized accumulator (frees the PSUM bank),
                # broadcast the denominator row across partitions on gpsimd,
                # then divide on DVE
                ao_ps = ao_tiles.pop((qh, h))
                ao_sb = nrm.tile([DH + 1, QH], F32, tag="ao_sb", name=f"aosb{i}")
                nc.vector.tensor_copy(ao_sb, ao_ps)
                cw = QH // nchunk
                stage_ts = [
                    osb.tile([P, 4, D], F32, tag="stg", name=f"stg{i}_{s}", bufs=2)
                    for s in range(nchunk // 4)
                ] if nchunk > 1 else []
                # stage the denominator row onto partition 0: the fast
                # 1-cyc/elem custom-DVE reciprocal reads partition 0 only
                # (standard reciprocal is ~7 cyc/elem)
                dn = nrm.tile([1, QH], F32, tag="dn", name=f"dn{i}", bufs=2)
                nc.vector.tensor_copy(dn, ao_sb[DH : DH + 1, :])
                for ch in range(nchunk):
                    cs = ch * cw
                    recip = nrm.tile([1, QH], F32, tag="rc", name=f"rc{i}_{ch}", bufs=2)
                    nc.vector.reciprocal_approx_fast(
                        out=recip[:, 0:cw], in_=dn[0:1, cs : cs + cw]
                    )
                    rb = nrm.tile([DH, QH], F32, tag="rb", name=f"rb{i}_{ch}", bufs=2)
                    nc.gpsimd.partition_broadcast(
                        rb[:, 0:cw], recip[:, 0:cw], channels=DH
                    )
                    nc.vector.tensor_tensor(
                        out=aoT[j][po : po + DH, q0 + cs : q0 + cs + cw],
                        in0=ao_sb[0:DH, cs : cs + cw],
                        in1=rb[:, 0:cw],
                        op=ALU.mult,
                    )
                    if nchunk > 1:
                        st = stage_ts[ch // 4]
                        outproj_tile(NT // 2 + ch, stage=(st, ch % 4))

            def attn_v(i):
                qh, h, kb = items[i]
                j, po = h // 2, (h % 2) * DH
                q0 = qh * QH
                if kb == 0:
                    ao_tiles[(qh, h)] = aop.tile(
                        [DH + 1, QH], F32, tag="ao", name=f"ao{qh}_{h}"
                    )
                ao_ps = ao_tiles[(qh, h)]
                ex = ex_tiles.pop(i)
                for c in range(2):
                    mm = nc.tensor.matmul(
                        ao_ps[:, c * 512 : (c + 1) * 512],
                        lhsT=(v_sb[:, kb, h, :]),
                        rhs=(ex[:, c * 512 : (c + 1) * 512]),
                        start=(kb == 0),
                        stop=(kb == NT - 1),
                    )
                    if c == 1:
                        mm.ins.ldweights = False
                if kb == NT - 1:
                    normalize(i, qh, h, j, po, q0, 8 if i == len(items) - 1 else 1)

            def outproj_tile(mt, stage=None):
                ps = opp.tile([P, D], F32, tag="o", name=f"o{mt}")
                for kt in range(2):
                    nc.tensor.matmul(
                        ps,
                        lhsT=(aoT[kt][:, mt * P : (mt + 1) * P]),
                        rhs=(w_o_sb[:, kt, :]),
                        start=(kt == 0),
                        stop=(kt == 1),
                    )
                if stage is None:
                    ot = osb.tile([P, D], F32, tag="ot", name=f"ot{mt}")
                    nc.vector.tensor_copy(ot, ps)
                    nc.sync.dma_start(out=out[mt * P : (mt + 1) * P, :], in_=ot)
                else:
                    # stage 4 tiles, then one batched DMA (fewer sync-engine
                    # trigger serializations in the kernel tail)
                    st, si = stage
                    nc.vector.tensor_copy(st[:, si, :], ps)
                    if si == 3:
                        m0 = mt - 3
                        nc.sync.dma_start(
                            out=out[m0 * P : (m0 + 4) * P, :].rearrange(
                                "(t p) d -> p t d", p=P
                            ),
                            in_=st,
                        )

            def qk_chunk(w_sb, b_sb, dstT, j, nt):
                ps = opp.tile([P, 512], F32, tag="o", name=f"qkc{j}_{dstT is qT}_{nt}")
                for dt in range(DT):
                    nc.tensor.matmul(
                        ps,
                        lhsT=(w_sb[:, dt, j * P : (j + 1) * P]),
                        rhs=(yT[dt][:, nt * 512 : (nt + 1) * 512]),
                        start=(dt == 0),
                        stop=(dt == DT - 1),
                    )
                nc.vector.tensor_scalar(
                    out=dstT[j][:, nt * 512 : (nt + 1) * 512],
                    in0=ps, scalar1=b_sb[:, j : j + 1], scalar2=None, op0=ALU.add,
                )

            # item index -> extra PE work issued right after attn_v(i):
            # deferred QK chunks early, first out-projection half spread
            # after unit (0,3)'s normalize completes
            post = {}
            for n, w in enumerate(qk_work):
                post[3 * n] = ("qk", w)
            for mt in range(NT // 2):
                post[76 + 2 * mt] = ("op", mt)

            DEPTH = 2
            for i in range(min(DEPTH, len(items))):
                sc_exp(i)
            for i in range(len(items)):
                if i + DEPTH < len(items):
                    sc_exp(i + DEPTH)
                attn_v(i)
                extra = post.get(i)
                if extra is not None:
                    if extra[0] == "qk":
                        qk_chunk(*extra[1])
                    else:
                        outproj_tile(extra[1])

    nc.compile()
    return nc


_NC_CACHE = None
_LAST_RESULT = None


def kernel(x, ln_scale, ln_bias, w_qkv, w_out):
    global _NC_CACHE, _LAST_RESULT
    if _NC_CACHE is None:
        _NC_CACHE = build_kernel()
    nc = _NC_CACHE

    import ml_dtypes

    x = np.asarray(x, np.float32)
    w_eff = (np.asarray(ln_scale, np.float32)[:, None] * np.asarray(w_qkv, np.float32))
    b_row = np.asarray(ln_bias, np.float32) @ np.asarray(w_qkv, np.float32)
    w_eff = w_eff.astype(ml_dtypes.bfloat16)
    w_out = np.asarray(w_out, np.float32).astype(ml_dtypes.bfloat16)

    in_maps = []
    for c in range(8):
        b, g = c // 2, c % 2
        s = slice(FPC * g, FPC * g + FPC)
        ks = slice(512 + FPC * g, 512 + FPC * g + FPC)
        vs = slice(1024 + FPC * g, 1024 + FPC * g + FPC)
        in_maps.append(
            {
                "xb": np.ascontiguousarray(x[b]),
                "wq": np.ascontiguousarray(w_eff[:, s]),
                "wk": np.ascontiguousarray(w_eff[:, ks]),
                "wv": np.ascontiguousarray(w_eff[:, vs]),
                "wo": np.ascontiguousarray(w_out[s, :]),
                "bq": np.ascontiguousarray(b_row[s]),
                "bk": np.ascontiguousarray(b_row[ks]),
                "bv": np.ascontiguousarray(b_row[vs]),
            }
        )
    res = run_bass_kernel_spmd(nc, in_maps, core_ids=list(range(8)))
    _LAST_RESULT = res
    outs = [res.results[c]["out"] for c in range(8)]
    return np.stack([outs[2 * b] + outs[2 * b + 1] for b in range(B)]).astype(
        np.float32
    )


if __name__ == "__main__":
    xs = np.random.randn(B, N, D).astype(np.float32)
    o = kernel(
        x=xs,
        ln_scale=np.ones(D, np.float32),
        ln_bias=np.zeros(D, np.float32),
        w_qkv=(np.random.randn(D, 3 * H * DH) / np.sqrt(D)).astype(np.float32),
        w_out=(np.random.randn(H * DH, D) / np.sqrt(H * DH)).astype(np.float32),
    )
    print(o.shape, o.dtype)


# revision 32
# speedup vs baseline: 1.0125x; 1.0074x over previous
"""Trainium2 Bass kernel for pre-LN multi-head self-attention.

Module: y = LN(x); qkv = y @ w_qkv; attention(8 heads, dh=64); out = ao @ w_out
Shapes: x [4, 2048, 512], w_qkv [512, 1536], w_out [512, 512], fp32.

Sharding (8 cores): core c -> batch b = c//2, head-group g = c%2 (4 heads).
Each core computes LN + QKV (its head slice) + attention + a partial output
projection (its heads' rows of w_out); the host sums the two partials per batch.

Design (v2, ACT-exp-stream centric):
  The softmax exp stream on the Scalar/ACT engine (16.8M elems/core at
  1 elem/cycle/lane @1.2GHz ~= 128us) is the hard floor; everything else is
  scheduled to keep that stream airtight and the PE clock warm (HAM K=8/8).
  - LN phase: 16-deep x-tile DMA lookahead; rstd = exp(-0.5*ln(var+eps)) so
    the whole kernel uses ONE ACT table set (natural_log_exp); y-affine on
    ACT, PSUM evictions on DVE; V-projection matmuls pipelined per token
    group to keep the PE busy during LN.
  - QK projections use [128,1024] PSUM accumulators from the same pool that
    later serves the score tiles; the j=1 head-pair projections are
    interleaved into early stage D so exps start right after j=0.
  - Stage D: depth-2 software pipeline (scores i+2 issue before attn@V i);
    single ao accumulator (eviction hides under the next unit's exp latency);
    softmax denominators (ones-column of V) broadcast across partitions via
    gpsimd.partition_broadcast and applied with a DVE divide -- no DRAM
    roundtrip, no 1-partition reciprocals.
  - Output projection tiles are spread one-per-item into PE slack; the last
    unit normalizes in 128-col chunks interleaved with the final tiles.
"""

import sys

if "/opt/trn_rl_repo" not in sys.path:
    sys.path.insert(0, "/opt/trn_rl_repo")

from contextlib import ExitStack

import numpy as np

import concourse.bass as bass
import concourse.tile as tile
from concourse.masks import make_identity
from concourse import bacc, mybir
from concourse.bass_utils import run_bass_kernel_spmd

B, N, D = 4, 2048, 512
H, DH = 8, 64
HPC = 4                 # heads per core
FPC = HPC * DH          # 256 features per core
P = 128
NT = N // P             # 16 token tiles
DT = D // P             # 4 d tiles
EPS = 1e-6
SCALE = DH ** -0.5
F32 = mybir.dt.float32
BF16 = mybir.dt.bfloat16
ALU = mybir.AluOpType
AFT = mybir.ActivationFunctionType
QH = 1024               # q-half width (stage D unit = (qh, h))


def build_kernel():
    nc = bacc.Bacc("TRN2", target_bir_lowering=False, debug=False)
    xb = nc.dram_tensor("xb", [N, D], F32, kind="ExternalInput").ap()
    wq = nc.dram_tensor("wq", [D, FPC], BF16, kind="ExternalInput").ap()
    wk = nc.dram_tensor("wk", [D, FPC], BF16, kind="ExternalInput").ap()
    wv = nc.dram_tensor("wv", [D, FPC], BF16, kind="ExternalInput").ap()
    wo = nc.dram_tensor("wo", [FPC, D], BF16, kind="ExternalInput").ap()
    bq = nc.dram_tensor("bq", [FPC], F32, kind="ExternalInput").ap()
    bk = nc.dram_tensor("bk", [FPC], F32, kind="ExternalInput").ap()
    bv = nc.dram_tensor("bv", [FPC], F32, kind="ExternalInput").ap()
    out = nc.dram_tensor("out", [N, D], F32, kind="ExternalOutput").ap()

    with tile.TileContext(nc, pool_alloc_mode="queue") as tc, ExitStack() as ctx:
        consts = ctx.enter_context(tc.tile_pool(name="consts", bufs=1))
        big = ctx.enter_context(tc.tile_pool(name="big", bufs=1))

        identity = consts.tile([P, P], BF16)
        make_identity(nc, identity)
        eps_t = consts.tile([P, 1], F32)
        nc.vector.memset(eps_t, EPS)

        yT = [big.tile([P, N], BF16, tag=f"yT{j}", name=f"yT{j}") for j in range(DT)]
        qT = [big.tile([P, N], BF16, tag=f"qT{j}", name=f"qT{j}") for j in range(2)]
        kT = [big.tile([P, N], BF16, tag=f"kT{j}", name=f"kT{j}") for j in range(2)]
        aoT = [big.tile([P, N], BF16, tag=f"aoT{j}", name=f"aoT{j}") for j in range(2)]
        v_sb = big.tile([P, NT, HPC, DH + 1], BF16)
        ones_col = consts.tile([P, 1], F32)
        nc.vector.memset(ones_col, 1.0)
        nc.vector.tensor_copy(
            v_sb[:, :, :, DH : DH + 1],
            ones_col[:, 0:1].to_broadcast((P, NT, HPC, 1)),
        )

        # ---- input + weight DMAs: x in 4 batched group DMAs (one trigger
        # each -- the sync engine serializes triggers at ~600ns apiece)
        xin = ctx.enter_context(tc.tile_pool(name="xin", bufs=4))
        x_gs = []
        for ig in range(4):
            x_g = xin.tile([P, 4, D], F32, tag="xg", name=f"xg{ig}")
            if ig == 0:
                # split the first group so tile 0 lands fast (small trigger)
                nc.sync.dma_start(
                    out=x_g[:, 0:1, :],
                    in_=xb[0:P, :].rearrange("(t p) d -> p t d", p=P),
                )
                nc.sync.dma_start(
                    out=x_g[:, 1:4, :],
                    in_=xb[P : 4 * P, :].rearrange("(t p) d -> p t d", p=P),
                )
            else:
                nc.sync.dma_start(
                    out=x_g,
                    in_=xb[ig * 512 : (ig + 1) * 512, :].rearrange(
                        "(t p) d -> p t d", p=P
                    ),
                )
            x_gs.append(x_g)
            if ig == 0:
                w_v_sb = consts.tile([P, DT, FPC], BF16)
                nc.sync.dma_start(
                    out=w_v_sb, in_=wv.rearrange("(t p) f -> p t f", p=P)
                )
                bv_b = consts.tile([P, FPC], F32)
                bv_bcast = bass.AP(
                    tensor=bv.tensor, offset=bv.offset, ap=[[0, P]] + list(bv.ap)
                )
                nc.sync.dma_start(out=bv_b, in_=bv_bcast)
            if ig == 1:
                w_q_sb = consts.tile([P, DT, FPC], BF16)
                nc.sync.dma_start(
                    out=w_q_sb, in_=wq.rearrange("(t p) f -> p t f", p=P)
                )
                w_k_sb = consts.tile([P, DT, FPC], BF16)
                nc.sync.dma_start(
                    out=w_k_sb, in_=wk.rearrange("(t p) f -> p t f", p=P)
                )
            if ig == 2:
                bq_sb = consts.tile([P, 2], F32)
                nc.sync.dma_start(out=bq_sb, in_=bq.rearrange("(t p) -> p t", p=P))
                bk_sb = consts.tile([P, 2], F32)
                nc.sync.dma_start(out=bk_sb, in_=bk.rearrange("(t p) -> p t", p=P))
                w_o_sb = consts.tile([P, 2, D], BF16)
                nc.sync.dma_start(
                    out=w_o_sb, in_=wo.rearrange("(t p) f -> p t f", p=P)
                )

        # bigp serves the j=0 QK accumulators and the stage-D score tiles
        bigp = ctx.enter_context(
            tc.tile_pool(name="bigp", bufs=2, space="PSUM")
        )

        def qk_half(w_sb, b_sb, dstT, j, half, on_act):
            ps = bigp.tile([P, QH], F32, tag="mm", name=f"qk{j}_{half}_{dstT is qT}")
            for dt in range(DT):
                for c in range(2):
                    mm = nc.tensor.matmul(
                        ps[:, c * 512 : (c + 1) * 512],
                        lhsT=(w_sb[:, dt, j * P : (j + 1) * P]),
                        rhs=(yT[dt][:, half * QH + c * 512 : half * QH + (c + 1) * 512]),
                        start=(dt == 0),
                        stop=(dt == DT - 1),
                    )
                    if c == 1:
                        mm.ins.ldweights = False
            cols = slice(half * QH, (half + 1) * QH)
            if on_act:
                nc.scalar.activation(
                    out=dstT[j][:, cols], in_=ps, func=AFT.Identity,
                    bias=b_sb[:, j : j + 1],
                )
            else:
                nc.vector.tensor_scalar(
                    out=dstT[j][:, cols], in0=ps, scalar1=b_sb[:, j : j + 1],
                    scalar2=None, op0=ALU.add,
                )

        # ---- Phase A: LayerNorm + transpose + V projection, pipelined;
        # the j=0 half-0 Q/K projections fire as soon as groups 0/1 land ----
        with tc.tile_pool(name="ln", bufs=8) as ln, tc.tile_pool(
            name="tp_psum", bufs=2, space="PSUM"
        ) as tpp, tc.tile_pool(name="v_psum", bufs=2, space="PSUM") as vpp:
            # dep-free PE warmup: dummy matmuls so the HAM clock-gate opens
            # (K=8/8) before the real prefix work arrives
            dmy = ln.tile([P, 512], BF16, tag="dmy")
            nc.vector.memset(dmy, 0.0)
            for k in range(24):
                dps = vpp.tile([P, FPC], F32, tag="v", name=f"dmy{k}")
                nc.tensor.matmul(
                    dps, lhsT=identity, rhs=dmy[:, 0:FPC], start=True, stop=True
                )
            for ig in range(NT // 4):  # groups of 4 token tiles
                y_ts = []
                for ii in range(4):
                    i = ig * 4 + ii
                    x_t = x_gs[ig][:, ii, :]
                    stats = ln.tile([P, 6], F32, tag="stats")
                    nc.vector.bn_stats(out=stats, in_=x_t)
                    mv = ln.tile([P, 2], F32, tag="mv")
                    nc.vector.bn_aggr(out=mv, in_=stats)
                    std = ln.tile([P, 1], F32, tag="std")
                    nc.scalar.activation(
                        out=std, in_=mv[:, 1:2], func=AFT.Sqrt, bias=eps_t[:, 0:1]
                    )
                    rstd = ln.tile([P, 1], F32, tag="rstd")
                    nc.vector.reciprocal(out=rstd, in_=std)
                    nmr = ln.tile([P, 1], F32, tag="nmr")
                    nc.vector.tensor_scalar(
                        out=nmr,
                        in0=mv[:, 0:1],
                        scalar1=rstd[:, 0:1],
                        scalar2=-1.0,
                        op0=ALU.mult,
                        op1=ALU.mult,
                    )
                    y_t = ln.tile([P, D], BF16, tag="y", bufs=6)
                    nc.scalar.activation(
                        out=y_t,
                        in_=x_t,
                        func=AFT.Identity,
                        scale=rstd[:, 0:1],
                        bias=nmr[:, 0:1],
                    )
                    y_ts.append(y_t)
                for j in range(DT):
                    pt = tpp.tile([P, 512], BF16, tag="tp")
                    for ii in range(4):
                        nc.tensor.transpose(
                            pt[:, ii * P : (ii + 1) * P],
                            y_ts[ii][:, j * P : (j + 1) * P],
                            identity,
                        )
                    nc.vector.tensor_copy(
                        yT[j][:, ig * 512 : (ig + 1) * 512], pt
                    )
                # V projection for this group's 4 token tiles
                for ii in range(4):
                    i = ig * 4 + ii
                    ps = vpp.tile([P, FPC], F32, tag="v", name=f"v{i}")
                    for dt in range(DT):
                        nc.tensor.matmul(
                            ps,
                            lhsT=(yT[dt][:, i * P : (i + 1) * P]),
                            rhs=(w_v_sb[:, dt, :]),
                            start=(dt == 0),
                            stop=(dt == DT - 1),
                        )
                    nc.vector.tensor_tensor(
                        out=v_sb[:, i, :, 0:DH],
                        in0=ps.rearrange("p (h d) -> p h d", h=HPC),
                        in1=bv_b.rearrange("p (h d) -> p h d", h=HPC),
                        op=ALU.add,
                    )
                if ig == 1:
                    # token half 0 of yT is complete: fire the j=0 half-0
                    # Q/K projections so stage D can start ASAP
                    qk_half(w_k_sb, bk_sb, kT, 0, 0, on_act=True)
                    qk_half(w_q_sb, bq_sb, qT, 0, 0, on_act=True)

        # remaining projection work in [128,512] chunks through the o_psum
        # pool (idle until the first out-projection), interleaved into early
        # stage D: j=0 half-1 first (needed at kb>=8), then j=1 (heads 2/3)
        qk_work = (
            [(w_k_sb, bk_sb, kT, 0, nt) for nt in (2, 3)]
            + [(w_k_sb, bk_sb, kT, 1, nt) for nt in range(4)]
            + [(w_q_sb, bq_sb, qT, 1, nt) for nt in range(4)]
            + [(w_q_sb, bq_sb, qT, 0, nt) for nt in (2, 3)]
        )

        # ---- Stage D ----
        with tc.tile_pool(name="ao_psum", bufs=1, space="PSUM") as aop, tc.tile_pool(
            name="exp_sb", bufs=6
        ) as exps, tc.tile_pool(name="nrm", bufs=3) as nrm, tc.tile_pool(
            name="o_psum", bufs=2, space="PSUM"
        ) as opp, tc.tile_pool(name="o_sb", bufs=3) as osb:
            items = [
                (qh, h, kb) for qh in range(2) for h in range(HPC) for kb in range(NT)
            ]
            ex_tiles = {}
            ao_tiles = {}

            def sc_exp(i):
                qh, h, kb = items[i]
                j, po = h // 2, (h % 2) * DH
                q0 = qh * QH
                sc = bigp.tile([P, QH], F32, tag="mm", name=f"sc{i}")
                for c in range(2):
                    mm = nc.tensor.matmul(
                        sc[:, c * 512 : (c + 1) * 512],
                        lhsT=(kT[j][po : po + DH, kb * P : (kb + 1) * P]),
                        rhs=(qT[j][po : po + DH, q0 + c * 512 : q0 + (c + 1) * 512]),
                        start=True,
                        stop=True,
                    )
                    if c == 1:
                        mm.ins.ldweights = False
                ex = exps.tile([P, QH], BF16, tag="ex", name=f"ex{i}")
                nc.scalar.activation(out=ex, in_=sc, func=AFT.Exp, scale=SCALE)
                ex_tiles[i] = ex

            def normalize(i, qh, h, j, po, q0, nchunk):
                # evict the unnormalized accumulator (frees the PSUM bank),
                # broadcast the denominator row across partitions on gpsimd,
                # then divide on DVE
                ao_ps = ao_tiles.pop((qh, h))
                ao_sb = nrm.tile([DH + 1, QH], F32, tag="ao_sb", name=f"aosb{i}")
                nc.vector.tensor_copy(ao_sb, ao_ps)
                cw = QH // nchunk
                stage_ts = [
                    osb.tile([P, 4, D], F32, tag="stg", name=f"stg{i}_{s}", bufs=2)
                    for s in range(nchunk // 4)
                ] if nchunk > 1 else []
                # stage the denominator row onto partition 0: the fast
                # 1-cyc/elem custom-DVE reciprocal reads partition 0 only
                # (standard reciprocal is ~7 cyc/elem)
                dn = nrm.tile([1, QH], F32, tag="dn", name=f"dn{i}", bufs=2)
                nc.vector.tensor_copy(dn, ao_sb[DH : DH + 1, :])
                for ch in range(nchunk):
                    cs = ch * cw
                    recip = nrm.tile([1, QH], F32, tag="rc", name=f"rc{i}_{ch}", bufs=2)
                    nc.vector.reciprocal_approx_fast(
                        out=recip[:, 0:cw], in_=dn[0:1, cs : cs + cw]
                    )
                    rb = nrm.tile([DH, QH], F32, tag="rb", name=f"rb{i}_{ch}", bufs=2)
                    nc.gpsimd.partition_broadcast(
                        rb[:, 0:cw], recip[:, 0:cw], channels=DH
                    )
                    nc.vector.tensor_tensor(
                        out=aoT[j][po : po + DH, q0 + cs : q0 + cs + cw],
                        in0=ao_sb[0:DH, cs : cs + cw],
                        in1=rb[:, 0:cw],
                        op=ALU.mult,
                    )
                    if nchunk > 1:
                        st = stage_ts[ch // 4]
                        outproj_tile(NT // 2 + ch, stage=(st, ch % 4))

            def attn_v(i):
                qh, h, kb = items[i]
                j, po = h // 2, (h % 2) * DH
                q0 = qh * QH
                if kb == 0:
                    ao_tiles[(qh, h)] = aop.tile(
                        [DH + 1, QH], F32, tag="ao", name=f"ao{qh}_{h}"
                    )
                ao_ps = ao_tiles[(qh, h)]
                ex = ex_tiles.pop(i)
                for c in range(2):
                    mm = nc.tensor.matmul(
                        ao_ps[:, c * 512 : (c + 1) * 512],
                        lhsT=(v_sb[:, kb, h, :]),
                        rhs=(ex[:, c * 512 : (c + 1) * 512]),
                        start=(kb == 0),
                        stop=(kb == NT - 1),
                    )
                    if c == 1:
                        mm.ins.ldweights = False
                if kb == NT - 1:
                    normalize(i, qh, h, j, po, q0, 8 if i == len(items) - 1 else 1)

            def outproj_tile(mt, stage=None):
                ps = opp.tile([P, D], F32, tag="o", name=f"o{mt}")
                for kt in range(2):
                    nc.tensor.matmul(
                        ps,
                        lhsT=(aoT[kt][:, mt * P : (mt + 1) * P]),
                        rhs=(w_o_sb[:, kt, :]),
                        start=(kt == 0),
                        stop=(kt == 1),
                    )
                if stage is None:
                    ot = osb.tile([P, D], F32, tag="ot", name=f"ot{mt}")
                    nc.vector.tensor_copy(ot, ps)
                    nc.sync.dma_start(out=out[mt * P : (mt + 1) * P, :], in_=ot)
                else:
                    # stage 4 tiles, then one batched DMA (fewer sync-engine
                    # trigger serializations in the kernel tail)
                    st, si = stage
                    nc.vector.tensor_copy(st[:, si, :], ps)
                    if si == 3:
                        m0 = mt - 3
                        nc.sync.dma_start(
                            out=out[m0 * P : (m0 + 4) * P, :].rearrange(
                                "(t p) d -> p t d", p=P
                            ),
                            in_=st,
                        )

            def qk_chunk(w_sb, b_sb, dstT, j, nt):
                ps = opp.tile([P, 512], F32, tag="o", name=f"qkc{j}_{dstT is qT}_{nt}")
                for dt in range(DT):
                    nc.tensor.matmul(
                        ps,
                        lhsT=(w_sb[:, dt, j * P : (j + 1) * P]),
                        rhs=(yT[dt][:, nt * 512 : (nt + 1) * 512]),
                        start=(dt == 0),
                        stop=(dt == DT - 1),
                    )
                nc.vector.tensor_scalar(
                    out=dstT[j][:, nt * 512 : (nt + 1) * 512],
                    in0=ps, scalar1=b_sb[:, j : j + 1], scalar2=None, op0=ALU.add,
                )

            # item index -> extra PE work issued right after attn_v(i):
            # deferred QK chunks early, first out-projection half spread
            # after unit (0,3)'s normalize completes
            post = {}
            for n, w in enumerate(qk_work):
                post[3 * n] = ("qk", w)
            for mt in range(NT // 2):
                post[76 + 2 * mt] = ("op", mt)

            DEPTH = 2
            for i in range(min(DEPTH, len(items))):
                sc_exp(i)
            for i in range(len(items)):
                if i + DEPTH < len(items):
                    sc_exp(i + DEPTH)
                attn_v(i)
                extra = post.get(i)
                if extra is not None:
                    if extra[0] == "qk":
                        qk_chunk(*extra[1])
                    else:
                        outproj_tile(extra[1])

    nc.compile()
    return nc


_NC_CACHE = None
_LAST_RESULT = None


def kernel(x, ln_scale, ln_bias, w_qkv, w_out):
    global _NC_CACHE, _LAST_RESULT
    if _NC_CACHE is None:
        _NC_CACHE = build_kernel()
    nc = _NC_CACHE

    import ml_dtypes

    x = np.asarray(x, np.float32)
    w_eff = (np.asarray(ln_scale, np.float32)[:, None] * np.asarray(w_qkv, np.float32))
    b_row = np.asarray(ln_bias, np.float32) @ np.asarray(w_qkv, np.float32)
    w_eff = w_eff.astype(ml_dtypes.bfloat16)
    w_out = np.asarray(w_out, np.float32).astype(ml_dtypes.bfloat16)

    in_maps = []
    for c in range(8):
        b, g = c // 2, c % 2
        s = slice(FPC * g, FPC * g + FPC)
        ks = slice(512 + FPC * g, 512 + FPC * g + FPC)
        vs = slice(1024 + FPC * g, 1024 + FPC * g + FPC)
        in_maps.append(
            {
                "xb": np.ascontiguousarray(x[b]),
                "wq": np.ascontiguousarray(w_eff[:, s]),
                "wk": np.ascontiguousarray(w_eff[:, ks]),
                "wv": np.ascontiguousarray(w_eff[:, vs]),
                "wo": np.ascontiguousarray(w_out[s, :]),
                "bq": np.ascontiguousarray(b_row[s]),
                "bk": np.ascontiguousarray(b_row[ks]),
                "bv": np.ascontiguousarray(b_row[vs]),
            }
        )
    res = run_bass_kernel_spmd(nc, in_maps, core_ids=list(range(8)))
    _LAST_RESULT = res
    outs = [res.results[c]["out"] for c in range(8)]
    return np.stack([outs[2 * b] + outs[2 * b + 1] for b in range(B)]).astype(
        np.float32
    )


if __name__ == "__main__":
    xs = np.random.randn(B, N, D).astype(np.float32)
    o = kernel(
        x=xs,
        ln_scale=np.ones(D, np.float32),
        ln_bias=np.zeros(D, np.float32),
        w_qkv=(np.random.randn(D, 3 * H * DH) / np.sqrt(D)).astype(np.float32),
        w_out=(np.random.randn(H * DH, D) / np.sqrt(H * DH)).astype(np.float32),
    )
    print(o.shape, o.dtype)


# revision 41
# speedup vs baseline: 1.0192x; 1.0066x over previous
"""Trainium2 Bass kernel for pre-LN multi-head self-attention.

Module: y = LN(x); qkv = y @ w_qkv; attention(8 heads, dh=64); out = ao @ w_out
Shapes: x [4, 2048, 512], w_qkv [512, 1536], w_out [512, 512], fp32.

Sharding (8 cores): core c -> batch b = c//2, head-group g = c%2 (4 heads).
Each core computes LN + QKV (its head slice) + attention + a partial output
projection (its heads' rows of w_out); the host sums the two partials per batch.

Design (ACT-exp-stream centric):
  The softmax exp stream on the Scalar/ACT engine (16.8M elems/core at
  1 elem/cycle/lane @1.2GHz ~= 143us incl per-instruction overhead) is the
  hard floor; everything else is scheduled to keep that stream airtight and
  the PE clock warm (HAM K=8/8):
  - x arrives via 4 batched group DMAs (single triggers; the sync engine
    serializes triggers at ~600ns each); weights follow the x stream.
  - LN: bn_stats/aggr pass for all 16 token tiles, then Sqrt (the only
    pre-exp ACT table set) + reciprocal, then the y-affine on DVE
    (tensor_scalar sub/mult) so the ACT queue drains early; PE transposes
    y -> yT; V projection matmuls keep the PE busy through the LN phase,
    and dep-free dummy matmuls warm the HAM clock-gate at the start.
  - The j=0 half-0 Q/K projections fire as soon as token half 0 of yT
    lands, gating stage D ~25us earlier than a fully sequential prefix;
    the remaining projection chunks run in the phase-A tail (j=0 half-1)
    and interleaved into early stage-D slots (j=1).
  - Stage D: depth-2 software pipeline (scores i+2 issue before attn@V i);
    score tiles, projection chunks and out-projection tiles share one
    2-buffer [128,1024] PSUM ring; double-buffered [65,1024] attn@V
    accumulators hide the unit-boundary eviction; softmax denominators
    (ones-column of V) are staged to partition 0, inverted with the fast
    custom-DVE reciprocal, broadcast across partitions on gpsimd
    (partition_broadcast), and applied with a DVE multiply -- no DRAM
    roundtrip, no slow 1-partition reciprocals.
  - Out-projection tiles are spread into PE slack after the first q-half
    completes; the last unit normalizes in 128-col chunks interleaved with
    the final tiles, whose outputs leave via 4-tile batched DMAs.
"""

import sys

if "/opt/trn_rl_repo" not in sys.path:
    sys.path.insert(0, "/opt/trn_rl_repo")

from contextlib import ExitStack

import numpy as np

import concourse.bass as bass
import concourse.tile as tile
from concourse.masks import make_identity
from concourse import bacc, mybir
from concourse.bass_utils import run_bass_kernel_spmd

B, N, D = 4, 2048, 512
H, DH = 8, 64
HPC = 4                 # heads per core
FPC = HPC * DH          # 256 features per core
P = 128
NT = N // P             # 16 token tiles
DT = D // P             # 4 d tiles
EPS = 1e-6
SCALE = DH ** -0.5
F32 = mybir.dt.float32
BF16 = mybir.dt.bfloat16
ALU = mybir.AluOpType
AFT = mybir.ActivationFunctionType
QH = 1024               # q-half width (stage D unit = (qh, h))


def build_kernel():
    nc = bacc.Bacc("TRN2", target_bir_lowering=False, debug=False)
    xb = nc.dram_tensor("xb", [N, D], F32, kind="ExternalInput").ap()
    wq = nc.dram_tensor("wq", [D, FPC], BF16, kind="ExternalInput").ap()
    wk = nc.dram_tensor("wk", [D, FPC], BF16, kind="ExternalInput").ap()
    wv = nc.dram_tensor("wv", [D, FPC], BF16, kind="ExternalInput").ap()
    wo = nc.dram_tensor("wo", [FPC, D], BF16, kind="ExternalInput").ap()
    bq = nc.dram_tensor("bq", [FPC], F32, kind="ExternalInput").ap()
    bk = nc.dram_tensor("bk", [FPC], F32, kind="ExternalInput").ap()
    bv = nc.dram_tensor("bv", [FPC], F32, kind="ExternalInput").ap()
    out = nc.dram_tensor("out", [N, D], F32, kind="ExternalOutput").ap()

    with tile.TileContext(nc, pool_alloc_mode="queue") as tc, ExitStack() as ctx:
        consts = ctx.enter_context(tc.tile_pool(name="consts", bufs=1))
        big = ctx.enter_context(tc.tile_pool(name="big", bufs=1))

        identity = consts.tile([P, P], BF16)
        make_identity(nc, identity)
        eps_t = consts.tile([P, 1], F32)
        nc.vector.memset(eps_t, EPS)

        yT = [big.tile([P, N], BF16, tag=f"yT{j}", name=f"yT{j}") for j in range(DT)]
        qT = [big.tile([P, N], BF16, tag=f"qT{j}", name=f"qT{j}") for j in range(2)]
        kT = [big.tile([P, N], BF16, tag=f"kT{j}", name=f"kT{j}") for j in range(2)]
        aoT = [big.tile([P, N], BF16, tag=f"aoT{j}", name=f"aoT{j}") for j in range(2)]
        v_sb = big.tile([P, NT, HPC, DH + 1], BF16)
        ones_col = consts.tile([P, 1], F32)
        nc.vector.memset(ones_col, 1.0)
        nc.vector.tensor_copy(
            v_sb[:, :, :, DH : DH + 1],
            ones_col[:, 0:1].to_broadcast((P, NT, HPC, 1)),
        )

        # ---- input + weight DMAs: x in 4 batched group DMAs (one trigger
        # each -- the sync engine serializes triggers at ~600ns apiece)
        xin = ctx.enter_context(tc.tile_pool(name="xin", bufs=4))
        x_gs = []
        for ig in range(4):
            x_g = xin.tile([P, 4, D], F32, tag="xg", name=f"xg{ig}")
            if ig == 0:
                # split the first group so tile 0 lands fast (small trigger)
                nc.sync.dma_start(
                    out=x_g[:, 0:1, :],
                    in_=xb[0:P, :].rearrange("(t p) d -> p t d", p=P),
                )
                nc.sync.dma_start(
                    out=x_g[:, 1:4, :],
                    in_=xb[P : 4 * P, :].rearrange("(t p) d -> p t d", p=P),
                )
            else:
                nc.sync.dma_start(
                    out=x_g,
                    in_=xb[ig * 512 : (ig + 1) * 512, :].rearrange(
                        "(t p) d -> p t d", p=P
                    ),
                )
            x_gs.append(x_g)
        # weights after x (x feeds the LN critical path; wo isn't needed
        # until the first out-projection)
        w_v_sb = consts.tile([P, DT, FPC], BF16)
        nc.sync.dma_start(out=w_v_sb, in_=wv.rearrange("(t p) f -> p t f", p=P))
        bv_b = consts.tile([P, FPC], F32)
        bv_bcast = bass.AP(
            tensor=bv.tensor, offset=bv.offset, ap=[[0, P]] + list(bv.ap)
        )
        nc.sync.dma_start(out=bv_b, in_=bv_bcast)
        w_q_sb = consts.tile([P, DT, FPC], BF16)
        nc.sync.dma_start(out=w_q_sb, in_=wq.rearrange("(t p) f -> p t f", p=P))
        w_k_sb = consts.tile([P, DT, FPC], BF16)
        nc.sync.dma_start(out=w_k_sb, in_=wk.rearrange("(t p) f -> p t f", p=P))
        bq_sb = consts.tile([P, 2], F32)
        nc.sync.dma_start(out=bq_sb, in_=bq.rearrange("(t p) -> p t", p=P))
        bk_sb = consts.tile([P, 2], F32)
        nc.sync.dma_start(out=bk_sb, in_=bk.rearrange("(t p) -> p t", p=P))
        w_o_sb = consts.tile([P, 2, D], BF16)
        nc.sync.dma_start(out=w_o_sb, in_=wo.rearrange("(t p) f -> p t f", p=P))

        # bigp serves the j=0 QK accumulators and the stage-D score tiles
        bigp = ctx.enter_context(
            tc.tile_pool(name="bigp", bufs=2, space="PSUM")
        )

        def qk_half(w_sb, b_sb, dstT, j, half, on_act):
            ps = bigp.tile([P, QH], F32, tag="mm", name=f"qk{j}_{half}_{dstT is qT}")
            for dt in range(DT):
                for c in range(2):
                    mm = nc.tensor.matmul(
                        ps[:, c * 512 : (c + 1) * 512],
                        lhsT=(w_sb[:, dt, j * P : (j + 1) * P]),
                        rhs=(yT[dt][:, half * QH + c * 512 : half * QH + (c + 1) * 512]),
                        start=(dt == 0),
                        stop=(dt == DT - 1),
                    )
                    if c == 1:
                        mm.ins.ldweights = False
            cols = slice(half * QH, (half + 1) * QH)
            if on_act:
                nc.scalar.activation(
                    out=dstT[j][:, cols], in_=ps, func=AFT.Identity,
                    bias=b_sb[:, j : j + 1],
                )
            else:
                nc.vector.tensor_scalar(
                    out=dstT[j][:, cols], in0=ps, scalar1=b_sb[:, j : j + 1],
                    scalar2=None, op0=ALU.add,
                )

        qk_chunk_early = []
        # ---- Phase A: LayerNorm + transpose + V projection, pipelined;
        # the j=0 half-0 Q/K projections fire as soon as groups 0/1 land ----
        with tc.tile_pool(name="ln", bufs=8) as ln, tc.tile_pool(
            name="tp_psum", bufs=2, space="PSUM"
        ) as tpp, tc.tile_pool(name="v_psum", bufs=2, space="PSUM") as vpp:
            # dep-free PE warmup: dummy matmuls so the HAM clock-gate opens
            # (K=8/8) before the real prefix work arrives
            dmy = ln.tile([P, 512], BF16, tag="dmy")
            nc.vector.memset(dmy, 0.0)
            for k in range(24):
                dps = vpp.tile([P, FPC], F32, tag="v", name=f"dmy{k}")
                nc.tensor.matmul(
                    dps, lhsT=identity, rhs=dmy[:, 0:FPC], start=True, stop=True
                )
            # pass 1: stats for all 16 tiles; pass 2: sqrt (the only ACT
            # work) + reciprocal -- keeps the ACT queue short so the exp
            # stream starts as soon as the half-0 projections land
            mvs, rstds = [], []
            for i in range(NT):
                x_t = x_gs[i // 4][:, i % 4, :]
                stats = ln.tile([P, 6], F32, tag="stats")
                nc.vector.bn_stats(out=stats, in_=x_t)
                mv = ln.tile([P, 2], F32, tag="mv", bufs=NT)
                nc.vector.bn_aggr(out=mv, in_=stats)
                mvs.append(mv)
            for i in range(NT):
                std = ln.tile([P, 1], F32, tag="std")
                nc.scalar.activation(
                    out=std, in_=mvs[i][:, 1:2], func=AFT.Sqrt, bias=eps_t[:, 0:1]
                )
                rstd = ln.tile([P, 1], F32, tag="rstd", bufs=NT)
                nc.vector.reciprocal(out=rstd, in_=std)
                rstds.append(rstd)
            # preload the exp ACT table set in idle prefix time so the
            # first stage-D exp doesn't pay the ~1.3us table swap
            pre_t = ln.tile([P, 1], F32, tag="pre")
            nc.scalar.activation(out=pre_t, in_=eps_t, func=AFT.Exp)

            def ln_group(ig):
                y_ts = []
                for ii in range(4):
                    i = ig * 4 + ii
                    x_t = x_gs[ig][:, ii, :]
                    y_t = ln.tile([P, D], BF16, tag="y", bufs=6)
                    nc.vector.tensor_scalar(
                        out=y_t,
                        in0=x_t,
                        scalar1=mvs[i][:, 0:1],
                        scalar2=rstds[i][:, 0:1],
                        op0=ALU.subtract,
                        op1=ALU.mult,
                    )
                    y_ts.append(y_t)
                for j in range(DT):
                    pt = tpp.tile([P, 512], BF16, tag="tp")
                    for ii in range(4):
                        nc.tensor.transpose(
                            pt[:, ii * P : (ii + 1) * P],
                            y_ts[ii][:, j * P : (j + 1) * P],
                            identity,
                        )
                    nc.vector.tensor_copy(
                        yT[j][:, ig * 512 : (ig + 1) * 512], pt
                    )

            def v_group(ig):
                for ii in range(4):
                    i = ig * 4 + ii
                    ps = vpp.tile([P, FPC], F32, tag="v", name=f"v{i}")
                    for dt in range(DT):
                        nc.tensor.matmul(
                            ps,
                            lhsT=(yT[dt][:, i * P : (i + 1) * P]),
                            rhs=(w_v_sb[:, dt, :]),
                            start=(dt == 0),
                            stop=(dt == DT - 1),
                        )
                    nc.vector.tensor_tensor(
                        out=v_sb[:, i, :, 0:DH],
                        in0=ps.rearrange("p (h d) -> p h d", h=HPC),
                        in1=bv_b.rearrange("p (h d) -> p h d", h=HPC),
                        op=ALU.add,
                    )

            # groups 0/1 LN+transpose, then the j=0 half-0 Q/K projections
            # immediately (the stage-D gate), V afterwards
            ln_group(0)
            ln_group(1)
            ps_k = bigp.tile([P, QH], F32, tag="mm", name="qk0k")
            for dt in range(DT):
                for c in range(2):
                    mm = nc.tensor.matmul(
                        ps_k[:, c * 512 : (c + 1) * 512],
                        lhsT=(w_k_sb[:, dt, 0:P]),
                        rhs=(yT[dt][:, c * 512 : (c + 1) * 512]),
                        start=(dt == 0),
                        stop=(dt == DT - 1),
                    )
                    if c == 1:
                        mm.ins.ldweights = False
            ps_q = bigp.tile([P, QH], F32, tag="mm", name="qk0q")
            for dt in range(DT):
                for c in range(2):
                    mm = nc.tensor.matmul(
                        ps_q[:, c * 512 : (c + 1) * 512],
                        lhsT=(w_q_sb[:, dt, 0:P]),
                        rhs=(yT[dt][:, c * 512 : (c + 1) * 512]),
                        start=(dt == 0),
                        stop=(dt == DT - 1),
                    )
                    if c == 1:
                        mm.ins.ldweights = False
            # evictions split and interleaved: the halves gating sc(0) first
            for cs, ps, b_sb, dstT in (
                (0, ps_k, bk_sb, kT), (0, ps_q, bq_sb, qT),
                (512, ps_k, bk_sb, kT), (512, ps_q, bq_sb, qT),
            ):
                nc.scalar.activation(
                    out=dstT[0][:, cs : cs + 512], in_=ps[:, cs : cs + 512],
                    func=AFT.Identity, bias=b_sb[:, 0:1],
                )
            v_group(0)
            v_group(1)
            ln_group(2)
            v_group(2)
            ln_group(3)
            v_group(3)
            # j=0 half-1 chunks here: they overlap the groups-2/3 LN tail
            # instead of punching gaps into the early exp stream
            for nt in (2, 3):
                qk_chunk_early.append((w_k_sb, bk_sb, kT, 0, nt))
                qk_chunk_early.append((w_q_sb, bq_sb, qT, 0, nt))

        # issue the collected j=0 half-1 chunks now (prefix tail)
        for (w_sb, b_sb, dstT, j, nt) in qk_chunk_early:
            ps = bigp.tile([P, QH], F32, tag="mm", name=f"qke{dstT is qT}_{nt}")[:, 0:512]
            for dt in range(DT):
                nc.tensor.matmul(
                    ps,
                    lhsT=(w_sb[:, dt, j * P : (j + 1) * P]),
                    rhs=(yT[dt][:, nt * 512 : (nt + 1) * 512]),
                    start=(dt == 0),
                    stop=(dt == DT - 1),
                )
            nc.vector.tensor_scalar(
                out=dstT[j][:, nt * 512 : (nt + 1) * 512],
                in0=ps, scalar1=b_sb[:, j : j + 1], scalar2=None, op0=ALU.add,
            )

        # j=1 (heads 2/3) projection chunks interleaved into early stage D
        qk_work = (
            [(w_k_sb, bk_sb, kT, 1, nt) for nt in range(4)]
            + [(w_q_sb, bq_sb, qT, 1, nt) for nt in range(4)]
        )

        # ---- Stage D ----
        with tc.tile_pool(name="ao_psum", bufs=2, space="PSUM") as aop, tc.tile_pool(
            name="exp_sb", bufs=4
        ) as exps, tc.tile_pool(name="nrm", bufs=3) as nrm, tc.tile_pool(
            name="o_sb", bufs=3
        ) as osb:
            items = [
                (qh, h, kb) for qh in range(2) for h in range(HPC) for kb in range(NT)
            ]
            ex_tiles = {}
            ao_tiles = {}

            def sc_exp(i):
                qh, h, kb = items[i]
                j, po = h // 2, (h % 2) * DH
                q0 = qh * QH
                sc = bigp.tile([P, QH], F32, tag="mm", name=f"sc{i}")
                for c in range(2):
                    mm = nc.tensor.matmul(
                        sc[:, c * 512 : (c + 1) * 512],
                        lhsT=(kT[j][po : po + DH, kb * P : (kb + 1) * P]),
                        rhs=(qT[j][po : po + DH, q0 + c * 512 : q0 + (c + 1) * 512]),
                        start=True,
                        stop=True,
                    )
                    if c == 1:
                        mm.ins.ldweights = False
                ex = exps.tile([P, QH], BF16, tag="ex", name=f"ex{i}")
                nc.scalar.activation(out=ex, in_=sc, func=AFT.Exp, scale=SCALE)
                ex_tiles[i] = ex

            def normalize(i, qh, h, j, po, q0, nchunk):
                # evict the unnormalized accumulator (frees the PSUM bank),
                # broadcast the denominator row across partitions on gpsimd,
                # then divide on DVE
                ao_ps = ao_tiles.pop((qh, h))
                ao_sb = nrm.tile([DH + 1, QH], F32, tag="ao_sb", name=f"aosb{i}")
                nc.vector.tensor_copy(ao_sb, ao_ps)
                cw = QH // nchunk
                stage_ts = [
                    osb.tile([P, 4, D], F32, tag="stg", name=f"stg{i}_{s}", bufs=2)
                    for s in range(nchunk // 4)
                ] if nchunk > 1 else []
                # stage the denominator row onto partition 0: the fast
                # 1-cyc/elem custom-DVE reciprocal reads partition 0 only
                # (standard reciprocal is ~7 cyc/elem)
                dn = nrm.tile([1, QH], F32, tag="dn", name=f"dn{i}", bufs=2)
                nc.vector.tensor_copy(dn, ao_sb[DH : DH + 1, :])
                for ch in range(nchunk):
                    cs = ch * cw
                    recip = nrm.tile([1, QH], F32, tag="rc", name=f"rc{i}_{ch}", bufs=2)
                    nc.vector.reciprocal_approx_fast(
                        out=recip[:, 0:cw], in_=dn[0:1, cs : cs + cw]
                    )
                    rb = nrm.tile([DH, QH], F32, tag="rb", name=f"rb{i}_{ch}", bufs=2)
                    nc.gpsimd.partition_broadcast(
                        rb[:, 0:cw], recip[:, 0:cw], channels=DH
                    )
                    nc.vector.tensor_tensor(
                        out=aoT[j][po : po + DH, q0 + cs : q0 + cs + cw],
                        in0=ao_sb[0:DH, cs : cs + cw],
                        in1=rb[:, 0:cw],
                        op=ALU.mult,
                    )
                    if nchunk > 1:
                        st = stage_ts[ch // 4]
                        outproj_tile(NT // 2 + ch, stage=(st, ch % 4))

            def attn_v(i):
                qh, h, kb = items[i]
                j, po = h // 2, (h % 2) * DH
                q0 = qh * QH
                if kb == 0:
                    ao_tiles[(qh, h)] = aop.tile(
                        [DH + 1, QH], F32, tag="ao", name=f"ao{qh}_{h}"
                    )
                ao_ps = ao_tiles[(qh, h)]
                ex = ex_tiles.pop(i)
                for c in range(2):
                    mm = nc.tensor.matmul(
                        ao_ps[:, c * 512 : (c + 1) * 512],
                        lhsT=(v_sb[:, kb, h, :]),
                        rhs=(ex[:, c * 512 : (c + 1) * 512]),
                        start=(kb == 0),
                        stop=(kb == NT - 1),
                    )
                    if c == 1:
                        mm.ins.ldweights = False
                if kb == NT - 1:
                    normalize(i, qh, h, j, po, q0, 8 if i == len(items) - 1 else 1)

            def outproj_tile(mt, stage=None):
                ps = bigp.tile([P, QH], F32, tag="mm", name=f"o{mt}")[:, 0:D]
                for kt in range(2):
                    nc.tensor.matmul(
                        ps,
                        lhsT=(aoT[kt][:, mt * P : (mt + 1) * P]),
                        rhs=(w_o_sb[:, kt, :]),
                        start=(kt == 0),
                        stop=(kt == 1),
                    )
                if stage is None:
                    ot = osb.tile([P, D], F32, tag="ot", name=f"ot{mt}")
                    nc.vector.tensor_copy(ot, ps)
                    nc.sync.dma_start(out=out[mt * P : (mt + 1) * P, :], in_=ot)
                else:
                    # stage 4 tiles, then one batched DMA (fewer sync-engine
                    # trigger serializations in the kernel tail)
                    st, si = stage
                    nc.vector.tensor_copy(st[:, si, :], ps)
                    if si == 3:
                        m0 = mt - 3
                        nc.sync.dma_start(
                            out=out[m0 * P : (m0 + 4) * P, :].rearrange(
                                "(t p) d -> p t d", p=P
                            ),
                            in_=st,
                        )

            def qk_chunk(w_sb, b_sb, dstT, j, nt):
                ps = bigp.tile([P, QH], F32, tag="mm", name=f"qkc{j}_{dstT is qT}_{nt}")[:, 0:512]
                for dt in range(DT):
                    nc.tensor.matmul(
                        ps,
                        lhsT=(w_sb[:, dt, j * P : (j + 1) * P]),
                        rhs=(yT[dt][:, nt * 512 : (nt + 1) * 512]),
                        start=(dt == 0),
                        stop=(dt == DT - 1),
                    )
                nc.vector.tensor_scalar(
                    out=dstT[j][:, nt * 512 : (nt + 1) * 512],
                    in0=ps, scalar1=b_sb[:, j : j + 1], scalar2=None, op0=ALU.add,
                )

            # item index -> extra PE work issued right after attn_v(i):
            # deferred QK chunks early, first out-projection half spread
            # after unit (0,3)'s normalize completes
            post = {}
            for n, w in enumerate(qk_work):
                post[3 * n] = ("qk", w)
            for mt in range(NT // 2):
                post[76 + 2 * mt] = ("op", mt)

            DEPTH = 2
            for i in range(min(DEPTH, len(items))):
                sc_exp(i)
            for i in range(len(items)):
                if i + DEPTH < len(items):
                    sc_exp(i + DEPTH)
                attn_v(i)
                extra = post.get(i)
                if extra is not None:
                    if extra[0] == "qk":
                        qk_chunk(*extra[1])
                    else:
                        outproj_tile(extra[1])

    nc.compile()
    return nc


_NC_CACHE = None
_LAST_RESULT = None


def kernel(x, ln_scale, ln_bias, w_qkv, w_out):
    global _NC_CACHE, _LAST_RESULT
    if _NC_CACHE is None:
        _NC_CACHE = build_kernel()
    nc = _NC_CACHE

    import ml_dtypes

    x = np.asarray(x, np.float32)
    w_eff = (np.asarray(ln_scale, np.float32)[:, None] * np.asarray(w_qkv, np.float32))
    b_row = np.asarray(ln_bias, np.float32) @ np.asarray(w_qkv, np.float32)
    w_eff = w_eff.astype(ml_dtypes.bfloat16)
    w_out = np.asarray(w_out, np.float32).astype(ml_dtypes.bfloat16)

    in_maps = []
    for c in range(8):
        b, g = c // 2, c % 2
        s = slice(FPC * g, FPC * g + FPC)
        ks = slice(512 + FPC * g, 512 + FPC * g + FPC)
        vs = slice(1024 + FPC * g, 1024 + FPC * g + FPC)
        in_maps.append(
            {
                "xb": np.ascontiguousarray(x[b]),
                "wq": np.ascontiguousarray(w_eff[:, s]),
                "wk": np.ascontiguousarray(w_eff[:, ks]),
                "wv": np.ascontiguousarray(w_eff[:, vs]),
                "wo": np.ascontiguousarray(w_out[s, :]),
                "bq": np.ascontiguousarray(b_row[s]),
                "bk": np.ascontiguousarray(b_row[ks]),
                "bv": np.ascontiguousarray(b_row[vs]),
            }
        )
    res = run_bass_kernel_spmd(nc, in_maps, core_ids=list(range(8)))
    _LAST_RESULT = res
    outs = [res.results[c]["out"] for c in range(8)]
    return np.stack([outs[2 * b] + outs[2 * b + 1] for b in range(B)]).astype(
        np.float32
    )


if __name__ == "__main__":
    xs = np.random.randn(B, N, D).astype(np.float32)
    o = kernel(
        x=xs,
        ln_scale=np.ones(D, np.float32),
        ln_bias=np.zeros(D, np.float32),
        w_qkv=(np.random.randn(D, 3 * H * DH) / np.sqrt(D)).astype(np.float32),
        w_out=(np.random.randn(H * DH, D) / np.sqrt(H * DH)).astype(np.float32),
    )
    print(o.shape, o.dtype)


# revision 43
# speedup vs baseline: 1.0209x; 1.0017x over previous
"""Trainium2 Bass kernel for pre-LN multi-head self-attention.

Module: y = LN(x); qkv = y @ w_qkv; attention(8 heads, dh=64); out = ao @ w_out
Shapes: x [4, 2048, 512], w_qkv [512, 1536], w_out [512, 512], fp32.

Sharding (8 cores): core c -> batch b = c//2, head-group g = c%2 (4 heads).
Each core computes LN + QKV (its head slice) + attention + a partial output
projection (its heads' rows of w_out); the host sums the two partials per batch.

Design (ACT-exp-stream centric):
  The softmax exp stream on the Scalar/ACT engine (16.8M elems/core at
  1 elem/cycle/lane @1.2GHz ~= 143us incl per-instruction overhead) is the
  hard floor; everything else is scheduled to keep that stream airtight and
  the PE clock warm (HAM K=8/8):
  - x arrives via 4 batched group DMAs (single triggers; the sync engine
    serializes triggers at ~600ns each); weights follow the x stream.
  - LN: bn_stats/aggr pass for all 16 token tiles, then Sqrt (the only
    pre-exp ACT table set) + reciprocal, then the y-affine on DVE
    (tensor_scalar sub/mult) so the ACT queue drains early; PE transposes
    y -> yT; V projection matmuls keep the PE busy through the LN phase,
    and dep-free dummy matmuls warm the HAM clock-gate at the start.
  - The j=0 half-0 Q/K projections fire as soon as token half 0 of yT
    lands, gating stage D ~25us earlier than a fully sequential prefix;
    the remaining projection chunks run in the phase-A tail (j=0 half-1)
    and interleaved into early stage-D slots (j=1).
  - Stage D: depth-2 software pipeline (scores i+2 issue before attn@V i);
    score tiles, projection chunks and out-projection tiles share one
    2-buffer [128,1024] PSUM ring; double-buffered [65,1024] attn@V
    accumulators hide the unit-boundary eviction; softmax denominators
    (ones-column of V) are staged to partition 0, inverted with the fast
    custom-DVE reciprocal, broadcast across partitions on gpsimd
    (partition_broadcast), and applied with a DVE multiply -- no DRAM
    roundtrip, no slow 1-partition reciprocals.
  - Out-projection tiles are spread into PE slack after the first q-half
    completes; the last unit normalizes in 128-col chunks interleaved with
    the final tiles, whose outputs leave via 4-tile batched DMAs.
"""

import sys

if "/opt/trn_rl_repo" not in sys.path:
    sys.path.insert(0, "/opt/trn_rl_repo")

from contextlib import ExitStack

import numpy as np

import concourse.bass as bass
import concourse.tile as tile
from concourse.masks import make_identity
from concourse import bacc, mybir
from concourse.bass_utils import run_bass_kernel_spmd

B, N, D = 4, 2048, 512
H, DH = 8, 64
HPC = 4                 # heads per core
FPC = HPC * DH          # 256 features per core
P = 128
NT = N // P             # 16 token tiles
DT = D // P             # 4 d tiles
EPS = 1e-6
SCALE = DH ** -0.5
F32 = mybir.dt.float32
BF16 = mybir.dt.bfloat16
ALU = mybir.AluOpType
AFT = mybir.ActivationFunctionType
QH = 1024               # q-half width (stage D unit = (qh, h))


def build_kernel():
    nc = bacc.Bacc("TRN2", target_bir_lowering=False, debug=False)
    xb = nc.dram_tensor("xb", [N, D], F32, kind="ExternalInput").ap()
    wq = nc.dram_tensor("wq", [D, FPC], BF16, kind="ExternalInput").ap()
    wk = nc.dram_tensor("wk", [D, FPC], BF16, kind="ExternalInput").ap()
    wv = nc.dram_tensor("wv", [D, FPC], BF16, kind="ExternalInput").ap()
    wo = nc.dram_tensor("wo", [FPC, D], BF16, kind="ExternalInput").ap()
    bq = nc.dram_tensor("bq", [FPC], F32, kind="ExternalInput").ap()
    bk = nc.dram_tensor("bk", [FPC], F32, kind="ExternalInput").ap()
    bv = nc.dram_tensor("bv", [FPC], F32, kind="ExternalInput").ap()
    out = nc.dram_tensor("out", [N, D], F32, kind="ExternalOutput").ap()

    with tile.TileContext(nc, pool_alloc_mode="queue") as tc, ExitStack() as ctx:
        consts = ctx.enter_context(tc.tile_pool(name="consts", bufs=1))
        big = ctx.enter_context(tc.tile_pool(name="big", bufs=1))

        identity = consts.tile([P, P], BF16)
        make_identity(nc, identity)
        eps_t = consts.tile([P, 1], F32)
        nc.vector.memset(eps_t, EPS)

        yT = [big.tile([P, N], BF16, tag=f"yT{j}", name=f"yT{j}") for j in range(DT)]
        qT = [big.tile([P, N], BF16, tag=f"qT{j}", name=f"qT{j}") for j in range(2)]
        kT = [big.tile([P, N], BF16, tag=f"kT{j}", name=f"kT{j}") for j in range(2)]
        aoT = [big.tile([P, N], BF16, tag=f"aoT{j}", name=f"aoT{j}") for j in range(2)]
        v_sb = big.tile([P, NT, HPC, DH + 1], BF16)
        ones_col = consts.tile([P, 1], F32)
        nc.vector.memset(ones_col, 1.0)
        nc.vector.tensor_copy(
            v_sb[:, :, :, DH : DH + 1],
            ones_col[:, 0:1].to_broadcast((P, NT, HPC, 1)),
        )

        # ---- input + weight DMAs: x in 4 batched group DMAs (one trigger
        # each -- the sync engine serializes triggers at ~600ns apiece)
        xin = ctx.enter_context(tc.tile_pool(name="xin", bufs=4))
        x_gs = []
        for ig in range(4):
            x_g = xin.tile([P, 4, D], F32, tag="xg", name=f"xg{ig}")
            if ig == 0:
                # split the first group so tile 0 lands fast (small trigger)
                nc.sync.dma_start(
                    out=x_g[:, 0:1, :],
                    in_=xb[0:P, :].rearrange("(t p) d -> p t d", p=P),
                )
                nc.sync.dma_start(
                    out=x_g[:, 1:4, :],
                    in_=xb[P : 4 * P, :].rearrange("(t p) d -> p t d", p=P),
                )
            else:
                nc.sync.dma_start(
                    out=x_g,
                    in_=xb[ig * 512 : (ig + 1) * 512, :].rearrange(
                        "(t p) d -> p t d", p=P
                    ),
                )
            x_gs.append(x_g)
        # weights after x (x feeds the LN critical path; wo isn't needed
        # until the first out-projection)
        w_v_sb = consts.tile([P, DT, FPC], BF16)
        nc.sync.dma_start(out=w_v_sb, in_=wv.rearrange("(t p) f -> p t f", p=P))
        bv_b = consts.tile([P, FPC], F32)
        bv_bcast = bass.AP(
            tensor=bv.tensor, offset=bv.offset, ap=[[0, P]] + list(bv.ap)
        )
        nc.sync.dma_start(out=bv_b, in_=bv_bcast)
        w_q_sb = consts.tile([P, DT, FPC], BF16)
        nc.sync.dma_start(out=w_q_sb, in_=wq.rearrange("(t p) f -> p t f", p=P))
        w_k_sb = consts.tile([P, DT, FPC], BF16)
        nc.sync.dma_start(out=w_k_sb, in_=wk.rearrange("(t p) f -> p t f", p=P))
        bq_sb = consts.tile([P, 2], F32)
        nc.sync.dma_start(out=bq_sb, in_=bq.rearrange("(t p) -> p t", p=P))
        bk_sb = consts.tile([P, 2], F32)
        nc.sync.dma_start(out=bk_sb, in_=bk.rearrange("(t p) -> p t", p=P))
        w_o_sb = consts.tile([P, 2, D], BF16)
        nc.sync.dma_start(out=w_o_sb, in_=wo.rearrange("(t p) f -> p t f", p=P))

        # bigp serves the j=0 QK accumulators and the stage-D score tiles
        bigp = ctx.enter_context(
            tc.tile_pool(name="bigp", bufs=2, space="PSUM")
        )

        def qk_half(w_sb, b_sb, dstT, j, half, on_act):
            ps = bigp.tile([P, QH], F32, tag="mm", name=f"qk{j}_{half}_{dstT is qT}")
            for dt in range(DT):
                for c in range(2):
                    mm = nc.tensor.matmul(
                        ps[:, c * 512 : (c + 1) * 512],
                        lhsT=(w_sb[:, dt, j * P : (j + 1) * P]),
                        rhs=(yT[dt][:, half * QH + c * 512 : half * QH + (c + 1) * 512]),
                        start=(dt == 0),
                        stop=(dt == DT - 1),
                    )
                    if c == 1:
                        mm.ins.ldweights = False
            cols = slice(half * QH, (half + 1) * QH)
            if on_act:
                nc.scalar.activation(
                    out=dstT[j][:, cols], in_=ps, func=AFT.Identity,
                    bias=b_sb[:, j : j + 1],
                )
            else:
                nc.vector.tensor_scalar(
                    out=dstT[j][:, cols], in0=ps, scalar1=b_sb[:, j : j + 1],
                    scalar2=None, op0=ALU.add,
                )

        qk_chunk_early = []
        # ---- Phase A: LayerNorm + transpose + V projection, pipelined;
        # the j=0 half-0 Q/K projections fire as soon as groups 0/1 land ----
        with tc.tile_pool(name="ln", bufs=8) as ln, tc.tile_pool(
            name="tp_psum", bufs=2, space="PSUM"
        ) as tpp, tc.tile_pool(name="v_psum", bufs=2, space="PSUM") as vpp:
            # dep-free PE warmup: dummy matmuls so the HAM clock-gate opens
            # (K=8/8) before the real prefix work arrives
            dmy = ln.tile([P, 512], BF16, tag="dmy")
            nc.vector.memset(dmy, 0.0)
            for k in range(24):
                dps = vpp.tile([P, FPC], F32, tag="v", name=f"dmy{k}")
                nc.tensor.matmul(
                    dps, lhsT=identity, rhs=dmy[:, 0:FPC], start=True, stop=True
                )
            # pass 1: stats for all 16 tiles; pass 2: sqrt (the only ACT
            # work) + reciprocal -- keeps the ACT queue short so the exp
            # stream starts as soon as the half-0 projections land
            mvs, rstds = [], []
            for i in range(NT):
                x_t = x_gs[i // 4][:, i % 4, :]
                stats = ln.tile([P, 6], F32, tag="stats")
                nc.vector.bn_stats(out=stats, in_=x_t)
                mv = ln.tile([P, 2], F32, tag="mv", bufs=NT)
                nc.vector.bn_aggr(out=mv, in_=stats)
                mvs.append(mv)
            for i in range(NT):
                std = ln.tile([P, 1], F32, tag="std")
                nc.scalar.activation(
                    out=std, in_=mvs[i][:, 1:2], func=AFT.Sqrt, bias=eps_t[:, 0:1]
                )
                rstd = ln.tile([P, 1], F32, tag="rstd", bufs=NT)
                nc.vector.reciprocal(out=rstd, in_=std)
                rstds.append(rstd)
            # preload the exp ACT table set during idle prefix time; reading
            # the last sqrt's output pins this AFTER all Sqrt-set work (the
            # scheduler would otherwise hoist it and cause a double reload),
            # so the first stage-D exp skips the ~1.3us table swap
            pre_t = ln.tile([P, 1], F32, tag="pre")
            nc.scalar.activation(out=pre_t, in_=std, func=AFT.Exp)

            def ln_group(ig):
                y_ts = []
                for ii in range(4):
                    i = ig * 4 + ii
                    x_t = x_gs[ig][:, ii, :]
                    y_t = ln.tile([P, D], BF16, tag="y", bufs=6)
                    nc.vector.tensor_scalar(
                        out=y_t,
                        in0=x_t,
                        scalar1=mvs[i][:, 0:1],
                        scalar2=rstds[i][:, 0:1],
                        op0=ALU.subtract,
                        op1=ALU.mult,
                    )
                    y_ts.append(y_t)
                for j in range(DT):
                    pt = tpp.tile([P, 512], BF16, tag="tp")
                    for ii in range(4):
                        nc.tensor.transpose(
                            pt[:, ii * P : (ii + 1) * P],
                            y_ts[ii][:, j * P : (j + 1) * P],
                            identity,
                        )
                    nc.vector.tensor_copy(
                        yT[j][:, ig * 512 : (ig + 1) * 512], pt
                    )

            def v_group(ig):
                for ii in range(4):
                    i = ig * 4 + ii
                    ps = vpp.tile([P, FPC], F32, tag="v", name=f"v{i}")
                    for dt in range(DT):
                        nc.tensor.matmul(
                            ps,
                            lhsT=(yT[dt][:, i * P : (i + 1) * P]),
                            rhs=(w_v_sb[:, dt, :]),
                            start=(dt == 0),
                            stop=(dt == DT - 1),
                        )
                    nc.vector.tensor_tensor(
                        out=v_sb[:, i, :, 0:DH],
                        in0=ps.rearrange("p (h d) -> p h d", h=HPC),
                        in1=bv_b.rearrange("p (h d) -> p h d", h=HPC),
                        op=ALU.add,
                    )

            # groups 0/1 LN+transpose, then the j=0 half-0 Q/K projections
            # immediately (the stage-D gate), V afterwards
            ln_group(0)
            ln_group(1)
            qk_half(w_k_sb, bk_sb, kT, 0, 0, on_act=True)
            qk_half(w_q_sb, bq_sb, qT, 0, 0, on_act=True)
            v_group(0)
            v_group(1)
            ln_group(2)
            v_group(2)
            ln_group(3)
            v_group(3)
            # j=0 half-1 chunks here: they overlap the groups-2/3 LN tail
            # instead of punching gaps into the early exp stream
            for nt in (2, 3):
                qk_chunk_early.append((w_k_sb, bk_sb, kT, 0, nt))
                qk_chunk_early.append((w_q_sb, bq_sb, qT, 0, nt))

        # issue the collected j=0 half-1 chunks now (prefix tail)
        for (w_sb, b_sb, dstT, j, nt) in qk_chunk_early:
            ps = bigp.tile([P, QH], F32, tag="mm", name=f"qke{dstT is qT}_{nt}")[:, 0:512]
            for dt in range(DT):
                nc.tensor.matmul(
                    ps,
                    lhsT=(w_sb[:, dt, j * P : (j + 1) * P]),
                    rhs=(yT[dt][:, nt * 512 : (nt + 1) * 512]),
                    start=(dt == 0),
                    stop=(dt == DT - 1),
                )
            nc.vector.tensor_scalar(
                out=dstT[j][:, nt * 512 : (nt + 1) * 512],
                in0=ps, scalar1=b_sb[:, j : j + 1], scalar2=None, op0=ALU.add,
            )

        # j=1 (heads 2/3) projection chunks interleaved into early stage D
        qk_work = (
            [(w_k_sb, bk_sb, kT, 1, nt) for nt in range(4)]
            + [(w_q_sb, bq_sb, qT, 1, nt) for nt in range(4)]
        )

        # ---- Stage D ----
        with tc.tile_pool(name="ao_psum", bufs=2, space="PSUM") as aop, tc.tile_pool(
            name="exp_sb", bufs=4
        ) as exps, tc.tile_pool(name="nrm", bufs=3) as nrm, tc.tile_pool(
            name="o_sb", bufs=3
        ) as osb:
            items = [
                (qh, h, kb) for qh in range(2) for h in range(HPC) for kb in range(NT)
            ]
            ex_tiles = {}
            ao_tiles = {}

            def sc_exp(i):
                qh, h, kb = items[i]
                j, po = h // 2, (h % 2) * DH
                q0 = qh * QH
                sc = bigp.tile([P, QH], F32, tag="mm", name=f"sc{i}")
                for c in range(2):
                    mm = nc.tensor.matmul(
                        sc[:, c * 512 : (c + 1) * 512],
                        lhsT=(kT[j][po : po + DH, kb * P : (kb + 1) * P]),
                        rhs=(qT[j][po : po + DH, q0 + c * 512 : q0 + (c + 1) * 512]),
                        start=True,
                        stop=True,
                    )
                    if c == 1:
                        mm.ins.ldweights = False
                ex = exps.tile([P, QH], BF16, tag="ex", name=f"ex{i}")
                nc.scalar.activation(out=ex, in_=sc, func=AFT.Exp, scale=SCALE)
                ex_tiles[i] = ex

            def normalize(i, qh, h, j, po, q0, nchunk):
                # evict the unnormalized accumulator (frees the PSUM bank),
                # broadcast the denominator row across partitions on gpsimd,
                # then divide on DVE
                ao_ps = ao_tiles.pop((qh, h))
                ao_sb = nrm.tile([DH + 1, QH], F32, tag="ao_sb", name=f"aosb{i}")
                nc.vector.tensor_copy(ao_sb, ao_ps)
                cw = QH // nchunk
                stage_ts = [
                    osb.tile([P, 4, D], F32, tag="stg", name=f"stg{i}_{s}", bufs=2)
                    for s in range(nchunk // 4)
                ] if nchunk > 1 else []
                # stage the denominator row onto partition 0: the fast
                # 1-cyc/elem custom-DVE reciprocal reads partition 0 only
                # (standard reciprocal is ~7 cyc/elem)
                dn = nrm.tile([1, QH], F32, tag="dn", name=f"dn{i}", bufs=2)
                nc.vector.tensor_copy(dn, ao_sb[DH : DH + 1, :])
                for ch in range(nchunk):
                    cs = ch * cw
                    recip = nrm.tile([1, QH], F32, tag="rc", name=f"rc{i}_{ch}", bufs=2)
                    nc.vector.reciprocal_approx_fast(
                        out=recip[:, 0:cw], in_=dn[0:1, cs : cs + cw]
                    )
                    rb = nrm.tile([DH, QH], F32, tag="rb", name=f"rb{i}_{ch}", bufs=2)
                    nc.gpsimd.partition_broadcast(
                        rb[:, 0:cw], recip[:, 0:cw], channels=DH
                    )
                    nc.vector.tensor_tensor(
                        out=aoT[j][po : po + DH, q0 + cs : q0 + cs + cw],
                        in0=ao_sb[0:DH, cs : cs + cw],
                        in1=rb[:, 0:cw],
                        op=ALU.mult,
                    )
                    if nchunk > 1:
                        st = stage_ts[ch // 4]
                        outproj_tile(NT // 2 + ch, stage=(st, ch % 4))

            def attn_v(i):
                qh, h, kb = items[i]
                j, po = h // 2, (h % 2) * DH
                q0 = qh * QH
                if kb == 0:
                    ao_tiles[(qh, h)] = aop.tile(
                        [DH + 1, QH], F32, tag="ao", name=f"ao{qh}_{h}"
                    )
                ao_ps = ao_tiles[(qh, h)]
                ex = ex_tiles.pop(i)
                for c in range(2):
                    mm = nc.tensor.matmul(
                        ao_ps[:, c * 512 : (c + 1) * 512],
                        lhsT=(v_sb[:, kb, h, :]),
                        rhs=(ex[:, c * 512 : (c + 1) * 512]),
                        start=(kb == 0),
                        stop=(kb == NT - 1),
                    )
                    if c == 1:
                        mm.ins.ldweights = False
                if kb == NT - 1:
                    normalize(i, qh, h, j, po, q0, 8 if i == len(items) - 1 else 1)

            def outproj_tile(mt, stage=None):
                ps = bigp.tile([P, QH], F32, tag="mm", name=f"o{mt}")[:, 0:D]
                for kt in range(2):
                    nc.tensor.matmul(
                        ps,
                        lhsT=(aoT[kt][:, mt * P : (mt + 1) * P]),
                        rhs=(w_o_sb[:, kt, :]),
                        start=(kt == 0),
                        stop=(kt == 1),
                    )
                if stage is None:
                    ot = osb.tile([P, D], F32, tag="ot", name=f"ot{mt}")
                    nc.vector.tensor_copy(ot, ps)
                    nc.sync.dma_start(out=out[mt * P : (mt + 1) * P, :], in_=ot)
                else:
                    # stage 4 tiles, then one batched DMA (fewer sync-engine
                    # trigger serializations in the kernel tail)
                    st, si = stage
                    nc.vector.tensor_copy(st[:, si, :], ps)
                    if si == 3:
                        m0 = mt - 3
                        nc.sync.dma_start(
                            out=out[m0 * P : (m0 + 4) * P, :].rearrange(
                                "(t p) d -> p t d", p=P
                            ),
                            in_=st,
                        )

            def qk_chunk(w_sb, b_sb, dstT, j, nt):
                ps = bigp.tile([P, QH], F32, tag="mm", name=f"qkc{j}_{dstT is qT}_{nt}")[:, 0:512]
                for dt in range(DT):
                    nc.tensor.matmul(
                        ps,
                        lhsT=(w_sb[:, dt, j * P : (j + 1) * P]),
                        rhs=(yT[dt][:, nt * 512 : (nt + 1) * 512]),
                        start=(dt == 0),
                        stop=(dt == DT - 1),
                    )
                nc.vector.tensor_scalar(
                    out=dstT[j][:, nt * 512 : (nt + 1) * 512],
                    in0=ps, scalar1=b_sb[:, j : j + 1], scalar2=None, op0=ALU.add,
                )

            # item index -> extra PE work issued right after attn_v(i):
            # deferred QK chunks early, first out-projection half spread
            # after unit (0,3)'s normalize completes
            post = {}
            for n, w in enumerate(qk_work):
                post[3 * n] = ("qk", w)
            for mt in range(NT // 2):
                post[76 + 2 * mt] = ("op", mt)

            DEPTH = 2
            for i in range(min(DEPTH, len(items))):
                sc_exp(i)
            for i in range(len(items)):
                if i + DEPTH < len(items):
                    sc_exp(i + DEPTH)
                attn_v(i)
                extra = post.get(i)
                if extra is not None:
                    if extra[0] == "qk":
                        qk_chunk(*extra[1])
                    else:
                        outproj_tile(extra[1])

    nc.compile()
    return nc


_NC_CACHE = None
_LAST_RESULT = None


def kernel(x, ln_scale, ln_bias, w_qkv, w_out):
    global _NC_CACHE, _LAST_RESULT
    if _NC_CACHE is None:
        _NC_CACHE = build_kernel()
    nc = _NC_CACHE

    import ml_dtypes

    x = np.asarray(x, np.float32)
    w_eff = (np.asarray(ln_scale, np.float32)[:, None] * np.asarray(w_qkv, np.float32))
    b_row = np.asarray(ln_bias, np.float32) @ np.asarray(w_qkv, np.float32)
    w_eff = w_eff.astype(ml_dtypes.bfloat16)
    w_out = np.asarray(w_out, np.float32).astype(ml_dtypes.bfloat16)

    in_maps = []
    for c in range(8):
        b, g = c // 2, c % 2
        s = slice(FPC * g, FPC * g + FPC)
        ks = slice(512 + FPC * g, 512 + FPC * g + FPC)
        vs = slice(1024 + FPC * g, 1024 + FPC * g + FPC)
        in_maps.append(
            {
                "xb": np.ascontiguousarray(x[b]),
                "wq": np.ascontiguousarray(w_eff[:, s]),
                "wk": np.ascontiguousarray(w_eff[:, ks]),
                "wv": np.ascontiguousarray(w_eff[:, vs]),
                "wo": np.ascontiguousarray(w_out[s, :]),
                "bq": np.ascontiguousarray(b_row[s]),
                "bk": np.ascontiguousarray(b_row[ks]),
                "bv": np.ascontiguousarray(b_row[vs]),
            }
        )
    res = run_bass_kernel_spmd(nc, in_maps, core_ids=list(range(8)))
    _LAST_RESULT = res
    outs = [res.results[c]["out"] for c in range(8)]
    return np.stack([outs[2 * b] + outs[2 * b + 1] for b in range(B)]).astype(
        np.float32
    )


if __name__ == "__main__":
    xs = np.random.randn(B, N, D).astype(np.float32)
    o = kernel(
        x=xs,
        ln_scale=np.ones(D, np.float32),
        ln_bias=np.zeros(D, np.float32),
        w_qkv=(np.random.randn(D, 3 * H * DH) / np.sqrt(D)).astype(np.float32),
        w_out=(np.random.randn(H * DH, D) / np.sqrt(H * DH)).astype(np.float32),
    )
    print(o.shape, o.dtype)
